# revision 23
# baseline (speedup 1.0000x reference)
"""CrossTemporalAttention2 Trainium2 kernel (v2).

Sharding: 8 cores = 2 batches x 4 query-chunks of 1024 rows. Each core runs
the full conv+LN+KV pipeline for its batch (duplicated within the batch
group) and attention + proj for its 1024 query rows.

v2 restructuring vs v1:
- LN folded algebraically: mean subtraction becomes a rank-1 matmul into the
  k/v projection PSUM accumulation; rstd becomes a per-partition scale
  applied inside exp (scores rows are m) / the v1 PSUM->SBUF copy (v1 rows
  are m).  Per-n score offsets cancel in softmax and are dropped; the v-side
  LN bias folds into the proj bias on the host.
- exp is split across engines: ACT (exact, with per-partition scale),
  DVE (4-op cubic minimax poly on [-0.95, 0.95]), GPSIMD (4-op cubic from a
  DVE-staged prescaled bf16 copy).  Scores live in [-0.71, 0.68].
- Emission order starts ACT exp as early as possible: q -> conv(x2) -> k2 ->
  all nh2=0 scores+exp -> conv(x1)+v1 -> nh2=0 U/pden -> normalize ->
  nh2=1 pipelined loop.
- PSUM budget (8 banks): cv[128,512]x2, scA[128,1024]x2, U0, U1; pden rides
  the cv ring after all phase-1 allocations.
"""

import numpy as np

B, N, C = 2, 4096, 256
H, Dh = 8, 32
M = 1024          # (64/2) * (64/2)
NCH = 1024        # query rows per core
SCALE = Dh ** -0.5
EPS = 1e-5

# minimax cubic fit of exp on [-0.95, 0.95] (rel err <= 4.1e-3 fp32)
C3, C2, C1, C0 = 0.15927659, 0.53526688, 1.00884709, 0.99703789

_prog_cache = {}


def _exp_engine(t):
    # t in [0, 64): tile index in emission order. Returns 'act'|'dve'.
    if t % 6 == 1:
        return "dve"
    return "act"


def _build_program():
    import concourse.bass as bass
    import concourse.bacc as bacc
    import concourse.tile as tile
    from concourse import mybir

    f32 = mybir.dt.float32
    bf16 = mybir.dt.bfloat16
    AF = mybir.ActivationFunctionType
    OP = mybir.AluOpType

    nc = bacc.Bacc()

    x1t = nc.dram_tensor("x1t", [C, N], bf16, kind="ExternalInput")
    x2t = nc.dram_tensor("x2t", [C, N], bf16, kind="ExternalInput")
    xqt = nc.dram_tensor("xqt", [C, NCH], bf16, kind="ExternalInput")
    w2d = nc.dram_tensor("w2", [2, 2, C, C], bf16, kind="ExternalInput")
    wall = nc.dram_tensor("wall", [C, 4 * C], bf16, kind="ExternalInput")
    rowd = nc.dram_tensor("rowd", [2, C], bf16, kind="ExternalInput")   # -kfcol, -vfcol
    cold = nc.dram_tensor("cold", [128, 4], f32, kind="ExternalInput")  # srb(2 oh), pb(2 oh)
    blkd = nc.dram_tensor("blkd", [2, 128, 128], bf16, kind="ExternalInput")
    eyed = nc.dram_tensor("eyed", [8, 8], f32, kind="ExternalInput")
    rstd = nc.dram_tensor("rstd", [2, M], f32, kind="Internal")
    outt = nc.dram_tensor("outt", [C, NCH], f32, kind="ExternalOutput")
    import os
    _dbg = os.environ.get("KDBG", "0") == "1"
    if _dbg:
        dbg_mu = nc.dram_tensor("dbg_mu", [2, M], f32, kind="ExternalOutput")
        dbg_rs = nc.dram_tensor("dbg_rs", [2, M], f32, kind="ExternalOutput")
        dbg_rc = nc.dram_tensor("dbg_rc", [2, 128, 8], f32, kind="ExternalOutput")
        dbg_k2 = nc.dram_tensor("dbg_k2", [128, 128], f32, kind="ExternalOutput")
        dbg_v1 = nc.dram_tensor("dbg_v1", [128, 128], f32, kind="ExternalOutput")
        dbg_q = nc.dram_tensor("dbg_q", [128, 128], f32, kind="ExternalOutput")
        dbg_et = nc.dram_tensor("dbg_et", [128, 1024], f32, kind="ExternalOutput")

    with nc.allow_low_precision(reason="bf16 matmul inputs; fp32 PSUM accumulation"), \
         tile.TileContext(nc) as tc:
      with tc.tile_pool(name="pg", bufs=1) as PG, \
           tc.tile_pool(name="psum", bufs=1, space="PSUM") as PS:
        # ================= consts / weights =================
        wallt = [PG.tile([128, 4 * C], bf16, name=f"wall{ch}",
                         tag=f"wall{ch}") for ch in range(2)]
        nc.gpsimd.dma_start(out=wallt[0], in_=wall[0:128, :])
        nc.sync.dma_start(out=wallt[1], in_=wall[128:256, :])
        wsb = {}
        for wi, nm in enumerate(("q", "k", "v", "p")):
            for ch in range(2):
                wsb[(nm, ch)] = wallt[ch][:, wi * C:(wi + 1) * C]
        kfneg = PG.tile([1, C], bf16, name="kfneg", tag="kfneg")
        nc.gpsimd.dma_start(out=kfneg, in_=rowd[0:1, :])
        vfneg = PG.tile([1, C], bf16, name="vfneg", tag="vfneg")
        nc.gpsimd.dma_start(out=vfneg, in_=rowd[1:2, :])
        cols = PG.tile([128, 4], f32, name="cols", tag="cols")
        nc.gpsimd.dma_start(out=cols, in_=cold[:])
        blk = []
        for grp in range(2):
            t = PG.tile([128, 128], bf16, name=f"blk{grp}", tag=f"blk{grp}")
            nc.gpsimd.dma_start(out=t, in_=blkd[grp])
            blk.append(t)
        w2 = []
        for ch in range(2):
            t = PG.tile([128, 2, 2, C], bf16, name=f"w2{ch}", tag=f"w2{ch}")
            nc.scalar.dma_start(
                out=t,
                in_=w2d[:, :, ch * 128:(ch + 1) * 128, :].rearrange(
                    "kh kw c o -> c kh kw o"))
            w2.append(t)
        xq = []
        for ch in range(2):
            t = PG.tile([128, NCH], bf16, name=f"xq{ch}", tag=f"xq{ch}")
            nc.scalar.dma_start(out=t, in_=xqt[ch * 128:(ch + 1) * 128, :])
            xq.append(t)
        # x loads split in N-halves so conv can start on the first half;
        # x2 (k2 path) first, spread over queues.
        xT = {}
        for inp, dram in ((1, x2t), (0, x1t)):
            for ch in range(2):
                t = PG.tile([128, N], bf16, name=f"x{inp}{ch}", tag=f"x{inp}{ch}")
                xT[(inp, ch)] = t
        for half in range(2):
            hs = slice(half * 2048, (half + 1) * 2048)
            nc.gpsimd.dma_start(out=xT[(1, 0)][:, hs], in_=x2t[0:128, hs])
            nc.sync.dma_start(out=xT[(1, 1)][:, hs], in_=x2t[128:256, hs])
        for half in range(2):
            hs = slice(half * 2048, (half + 1) * 2048)
            nc.gpsimd.dma_start(out=xT[(0, 0)][:, hs], in_=x1t[0:128, hs])
            nc.sync.dma_start(out=xT[(0, 1)][:, hs], in_=x1t[128:256, hs])
        # small consts after the bulk loads (gpsimd queue is in order)

        ones1 = PG.tile([1, 128], bf16, name="ones1", tag="ones1")
        nc.vector.memset(ones1, 1.0)
        selc = PG.tile([128, 1], bf16, name="selc", tag="selc")
        nc.vector.memset(selc, 1.0)
        sel8 = PG.tile([128, 2, 8], bf16, name="sel8", tag="sel8")
        nc.vector.memset(sel8, 0.0)
        for g in range(2):
            nc.vector.memset(sel8[:, g, g:g + 1], 1.0)
        eye8 = PG.tile([8, 8], f32, name="eye8", tag="eye8")
        nc.gpsimd.dma_start(out=eye8, in_=eyed[:])
        epsT = PG.tile([1, 1], f32, name="epsT", tag="epsT")
        nc.vector.memset(epsT, EPS)

        # ================= SBUF data tiles =================
        qT = [PG.tile([128, NCH], bf16, name=f"qT{oh}", tag=f"qT{oh}")
              for oh in range(2)]
        xr = {(inp, oh): PG.tile([128, M], bf16, name=f"xr{inp}{oh}",
                                 tag=f"xr{inp}{oh}")
              for inp in range(2) for oh in range(2)}
        # sq tiles shared between the two inputs (x2 stats finish before
        # conv(x1) writes them again; WAR handled by tile deps)
        sq = {}
        for inp in (1, 0):   # allocation order must match usage order
            for oh in range(2):
                sq[(inp, oh)] = PG.tile([128, M], bf16, name=f"sqt{oh}",
                                        tag=f"sqt{oh}", bufs=1)
        k2x = [PG.tile([128, M], bf16, name=f"k2x{oh}", tag=f"k2x{oh}")
               for oh in range(2)]
        v1 = [PG.tile([128, C], bf16, name=f"v1_{ms}", tag=f"v1_{ms}")
              for ms in range(8)]
        murow = [PG.tile([1, M], bf16, name=f"mu{inp}", tag=f"mu{inp}")
                 for inp in range(2)]
        varrow = [PG.tile([1, M], f32, name=f"va{inp}", tag=f"va{inp}")
                  for inp in range(2)]
        rcol = [PG.tile([128, 8], f32, name=f"rcol{inp}", tag=f"rcol{inp}")
                for inp in range(2)]

        def cvt():
            return PS.tile([128, 512], f32, name="cv", tag="cv", bufs=2)

        # ================= q =================
        for oh in range(2):
            for nh in range(2):
                ps = cvt()
                for ch in range(2):
                    nc.tensor.matmul(
                        ps, wsb[("q", ch)][:, oh * 128:(oh + 1) * 128],
                        xq[ch][:, nh * 512:(nh + 1) * 512],
                        start=(ch == 0), stop=(ch == 1))
                nc.scalar.copy(out=qT[oh][:, nh * 512:(nh + 1) * 512], in_=ps)

        # ================= conv + stats + proj-k/v helpers ============
        def conv_half(inp, mh):
            # conv for m-columns [mh*512, (mh+1)*512) of both oh chunks
            for oh in range(2):
                ps = cvt()
                k = 0
                for ch in range(2):
                    xv = xT[(inp, ch)].rearrange(
                        "p (i ki j kj) -> p ki kj i j", ki=2, kj=2, j=32)
                    for kh in range(2):
                        for kw in range(2):
                            nc.tensor.matmul(
                                ps,
                                w2[ch][:, kh, kw, oh * 128:(oh + 1) * 128],
                                xv[:, kh, kw, mh * 16:(mh + 1) * 16, :],
                                start=(k == 0), stop=(k == 7))
                            k += 1
                sl = slice(mh * 512, (mh + 1) * 512)
                # copy + per-partition conv bias (srb col 0/1)
                nc.vector.tensor_scalar(
                    out=xr[(inp, oh)][:, sl], in0=ps,
                    scalar1=cols[:, oh:oh + 1], scalar2=None, op0=OP.add)
                nc.gpsimd.tensor_mul(sq[(inp, oh)][:, sl],
                                     xr[(inp, oh)][:, sl],
                                     xr[(inp, oh)][:, sl])

        def stats_half(inp, mh):
            sl = slice(mh * 512, (mh + 1) * 512)
            pmu = PS.tile([1, 512], f32, name="pmu", tag="scA", bufs=2)
            psq = PS.tile([1, 512], f32, name="psq", tag="scA", bufs=2)
            for k, oh in enumerate(range(2)):
                nc.tensor.matmul(pmu, selc, xr[(inp, oh)][:, sl],
                                 start=(k == 0), stop=(k == 1))
                nc.tensor.matmul(psq, selc, sq[(inp, oh)][:, sl],
                                 start=(k == 0), stop=(k == 1))
            nc.scalar.mul(out=murow[inp][:, sl], in_=pmu, mul=1.0 / C)
            mu2 = PG.tile([1, 512], f32, name="mu2", tag="mu2", bufs=2)
            nc.vector.tensor_mul(mu2, murow[inp][:, sl], murow[inp][:, sl])
            nc.vector.scalar_tensor_tensor(
                out=varrow[inp][:, sl], in0=psq, scalar=1.0 / C, in1=mu2,
                op0=OP.mult, op1=OP.subtract)

        def rcol_make(inp):
            # var row [1, 1024] -> [128, 8] columns (DRAM bounce), then
            # rsqrt(var + eps) on DVE: bit-trick seed + 2 Newton steps.
            nc.gpsimd.dma_start(out=rstd[inp:inp + 1, :], in_=varrow[inp])
            vc = PG.tile([128, 8], f32, name="vc", tag="vc", bufs=2)
            nc.gpsimd.dma_start(
                out=vc,
                in_=rstd[inp:inp + 1, :].rearrange("o (j p) -> (o p) j",
                                                   p=128))
            nc.vector.tensor_scalar(out=vc, in0=vc, scalar1=float(EPS),
                                    scalar2=None, op0=OP.add)
            i32 = mybir.dt.int32
            sh = PG.tile([128, 8], i32, name="sh", tag="sh", bufs=2)
            nc.vector.tensor_scalar(out=sh, in0=vc.bitcast(i32), scalar1=1,
                                    scalar2=None, op0=OP.arith_shift_right)
            y0 = PG.tile([128, 8], i32, name="y0", tag="y0", bufs=2)
            nc.vector.tensor_scalar(out=y0, in0=sh, scalar1=-1,
                                    scalar2=0x5F3759DF,
                                    op0=OP.mult, op1=OP.add)
            y = y0.bitcast(f32)
            for it in range(2):
                c = PG.tile([128, 8], f32, name="nc1", tag="nc1", bufs=2)
                nc.vector.tensor_mul(c, y, y)
                nc.vector.tensor_mul(c, c, vc)
                nc.vector.tensor_scalar(out=c, in0=c, scalar1=-0.5,
                                        scalar2=1.5, op0=OP.mult, op1=OP.add)
                dst = rcol[inp] if it == 1 else y
                nc.vector.tensor_mul(dst, y, c)

        def k2x_half(mh):
            sl = slice(mh * 512, (mh + 1) * 512)
            for oh in range(2):
                ps = cvt()
                for ch in range(2):
                    nc.tensor.matmul(
                        ps, wsb[("k", ch)][:, oh * 128:(oh + 1) * 128],
                        xr[(1, ch)][:, sl], start=(ch == 0), stop=False)
                # rank-1: += (-kfcol) x mu2  (mean subtraction folded in)
                nc.tensor.matmul(
                    ps, kfneg[:, oh * 128:(oh + 1) * 128],
                    murow[1][:, sl], start=False, stop=True)
                nc.vector.tensor_copy(out=k2x[oh][:, sl], in_=ps)

        def v1_chunk(ms):
            msl = slice(ms * 128, (ms + 1) * 128)
            ps = PS.tile([128, C], f32, name="vp", tag="cv", bufs=2)
            for ch in range(2):
                nc.tensor.matmul(ps, xr[(0, ch)][:, msl], wsb[("v", ch)],
                                 start=(ch == 0), stop=False)
            # rank-1: += mu1 x (-vfcol)
            nc.tensor.matmul(ps, murow[0][:, msl], vfneg,
                             start=False, stop=True)
            # copy with per-partition rstd scale folded in
            nc.vector.tensor_scalar(
                out=v1[ms], in0=ps, scalar1=rcol[0][:, ms:ms + 1],
                scalar2=None, op0=OP.mult)

        # ================= phase 2 machinery =================
        ET = tc.alloc_tile_pool(name="et", bufs=(28 if _dbg else 34))
        XG = tc.alloc_tile_pool(name="xg", bufs=3)
        tile_ctr = [0]

        def scores_exp(nh2, ms, grp):
            """scores for 2 pr-tiles (4 heads) + exp on assigned engine.
            Returns the two et tiles."""
            nsl = slice(nh2 * 512, (nh2 + 1) * 512)
            ets = []
            for pr in range(2):
                t = tile_ctr[0]
                tile_ctr[0] += 1
                eng = _exp_engine(t)
                scps = PS.tile([128, 1024], f32, name="scps", tag="scA",
                               bufs=2)
                for i in range(2):
                    h = grp * 4 + pr * 2 + i
                    hb = 32 * (h % 4)
                    nc.tensor.matmul(
                        scps[:, i * 512:(i + 1) * 512],
                        k2x[h // 4][hb:hb + 32, ms * 128:(ms + 1) * 128],
                        qT[h // 4][hb:hb + 32, nsl],
                        start=True, stop=True,
                        tile_position=(hb, 0))
                et = ET.tile([128, 1024], bf16, name="et", tag="et")
                rsc = rcol[1][:, ms:ms + 1]
                if eng == "act":
                    nc.scalar.activation(out=et, in_=scps, func=AF.Exp,
                                         scale=rsc)
                else:  # dve cubic
                    t = XG.tile([128, 1024], bf16, name="t", tag="t", bufs=1)
                    nc.vector.tensor_scalar(
                        out=t, in0=scps, scalar1=rsc, scalar2=None,
                        op0=OP.mult)
                    u = XG.tile([128, 1024], bf16, name="u", tag="u", bufs=1)
                    nc.vector.tensor_mul(u, t, t)
                    v = XG.tile([128, 1024], bf16, name="v", tag="v", bufs=1)
                    nc.vector.tensor_scalar(
                        out=v, in0=t, scalar1=float(C3), scalar2=float(C2),
                        op0=OP.mult, op1=OP.add)
                    p = XG.tile([128, 1024], bf16, name="p", tag="p", bufs=1)
                    nc.vector.tensor_mul(p, u, v)
                    w = XG.tile([128, 1024], bf16, name="w", tag="w", bufs=1)
                    nc.vector.tensor_scalar(
                        out=w, in0=t, scalar1=float(C1), scalar2=float(C0),
                        op0=OP.mult, op1=OP.add)
                    nc.vector.tensor_add(et, w, p)
                ets.append(et)
            return ets

        U = {}
        pden = {}

        def upden(nh2, ms, grp, ets):
            if ms == 0 and grp == 0:
                U[(nh2, 0)] = PS.tile([128, 512], f32, name="U0", tag="U0")
                U[(nh2, 1)] = PS.tile([128, 512], f32, name="U1", tag="U1")
                pden[nh2] = PS.tile([128, 512], f32, name="pden", tag="cv",
                                    bufs=2)
            for pr in range(2):
                for i in range(2):
                    h = grp * 4 + pr * 2 + i
                    h4 = pr * 2 + i
                    esl = ets[pr][:, i * 512:(i + 1) * 512]
                    nc.tensor.matmul(
                        U[(nh2, grp)][32 * h4:32 * h4 + 32, :],
                        v1[ms][:, 32 * h:32 * h + 32], esl,
                        start=(ms == 0), stop=(ms == 7),
                        tile_position=(0, 32 * h4),
                        skip_group_check=True)
            for pr in range(2):
                for i in range(2):
                    h = grp * 4 + pr * 2 + i
                    g = h % 4
                    esl = ets[pr][:, i * 512:(i + 1) * 512]
                    nc.tensor.matmul(
                        pden[nh2][32 * g:32 * g + 8, :],
                        sel8[:, h // 4, :], esl,
                        start=(ms == 0 and grp == 0),
                        stop=(ms == 7 and grp == 1),
                        tile_position=(0, 32 * g),
                        skip_group_check=True)

        def normalize_proj(nh2):
            nsl = slice(nh2 * 512, (nh2 + 1) * 512)
            pdenS = PG.tile([128, 512], bf16, name="pdenS", tag="pdenS",
                            bufs=2)
            nc.vector.tensor_copy(out=pdenS, in_=pden[nh2])
            rps = PS.tile([128, 1024], f32, name="rps", tag="scA", bufs=2)
            for grp in range(2):
                nc.tensor.matmul(rps[:, grp * 512:(grp + 1) * 512],
                                 blk[grp], pdenS, start=True, stop=True)
            recf = PG.tile([128, 1024], f32, name="recf", tag="recf",
                           bufs=2)
            nc.vector.reciprocal_approx_fast(out=recf, in_=rps)
            oT = []
            for grp in range(2):
                ot = PG.tile([128, 512], bf16, name="ot", tag=f"ot{grp}",
                             bufs=2)
                nc.vector.tensor_mul(ot, U[(nh2, grp)],
                                     recf[:, grp * 512:(grp + 1) * 512])
                oT.append(ot)
            for oh in range(2):
                ps = cvt()
                for ch in range(2):
                    nc.tensor.matmul(
                        ps, wsb[("p", ch)][:, oh * 128:(oh + 1) * 128],
                        oT[ch], start=(ch == 0), stop=(ch == 1))
                y = PG.tile([128, 512], f32, name="y", tag="y", bufs=2)
                # copy + per-partition proj bias (pb col 2/3)
                nc.vector.tensor_scalar(
                    out=y, in0=ps, scalar1=cols[:, 2 + oh:3 + oh],
                    scalar2=None, op0=OP.add)
                nc.gpsimd.dma_start(out=outt[oh * 128:(oh + 1) * 128, nsl],
                                    in_=y)

        # ================= emission =================
        # phase 1a: conv(x2) + stats + k2x  (rcol as soon as stats done)
        conv_half(1, 0)
        stats_half(1, 0)
        conv_half(1, 1)
        stats_half(1, 1)
        rcol_make(1)
        k2x_half(0)
        k2x_half(1)

        # phase 2a: nh2=0 scores + exp, with phase 1b (conv(x1)+v1) steps
        # interleaved so the PE stays dense while ACT chews on exp.
        p1b_steps = (
            [lambda: conv_half(0, 0)] +
            [lambda: stats_half(0, 0)] +
            [lambda: conv_half(0, 1)] +
            [lambda: stats_half(0, 1), lambda: rcol_make(0)] +
            [lambda ms=ms: v1_chunk(ms) for ms in range(8)] +
            [lambda: None] * 3
        )
        ets0 = {}
        step = 0
        for ms in range(8):
            for grp in range(2):
                ets0[(ms, grp)] = scores_exp(0, ms, grp)
                p1b_steps[step]()
                step += 1

        if _dbg:
            for inp in range(2):
                mcf = PG.tile([1, M], f32, name="mcf", tag="mcf", bufs=1)
                nc.vector.tensor_copy(out=mcf, in_=murow[inp])
                nc.gpsimd.dma_start(out=dbg_mu[inp:inp+1, :], in_=mcf)
                nc.gpsimd.dma_start(out=dbg_rs[inp:inp+1, :], in_=rsrow[inp])
                nc.gpsimd.dma_start(out=dbg_rc[inp], in_=rcol[inp])
            smp = PG.tile([128, 128], f32, name="smp", tag="smp", bufs=2)
            for nmd, srcd in (("dbg_k2", k2x[0]), ("dbg_v1", v1[0]),
                              ("dbg_q", qT[0])):
                s = PG.tile([128, 128], f32, name="s_" + nmd, tag="smp",
                            bufs=2)
                nc.vector.tensor_copy(out=s, in_=srcd[:, 0:128])
                d = {"dbg_k2": dbg_k2, "dbg_v1": dbg_v1, "dbg_q": dbg_q}[nmd]
                nc.gpsimd.dma_start(out=d[:, 0:128], in_=s)
            etf = PG.tile([128, 1024], f32, name="etf", tag="etf")
            nc.vector.tensor_copy(out=etf, in_=ets0[(0, 0)][0])
            nc.gpsimd.dma_start(out=dbg_et[:], in_=etf)

        # phase 2b: nh2=0 U/pden
        for ms in range(8):
            for grp in range(2):
                upden(0, ms, grp, ets0[(ms, grp)])
        normalize_proj(0)

        # phase 2c: nh2=1, software pipelined at distance 2 (scA bufs=2
        # allows two score tiles in flight; U/pden trail by two iterations)
        pend = []
        for ms in range(8):
            for grp in range(2):
                ets = scores_exp(1, ms, grp)
                pend.append((ms, grp, ets))
                if len(pend) > 2:
                    upden(1, *pend.pop(0))
        for it in pend:
            upden(1, *it)
        normalize_proj(1)

        XG.release()
        ET.release()
    nc.finalize()
    return nc


def _get_program():
    if "nc" not in _prog_cache:
        _prog_cache["nc"] = _build_program()
    return _prog_cache["nc"]


def kernel(x1, x2, q_w, kv_w, sr_w, sr_b, ln_g, ln_b, proj_w, proj_b,
           H1=64, W1=64, H2=64, W2=64, **_):
    from concourse.bass_utils import run_bass_kernel_spmd

    f = np.float32
    x1 = np.asarray(x1, f)
    x2 = np.asarray(x2, f)
    q_w = np.asarray(q_w, f)
    kv_w = np.asarray(kv_w, f)
    sr_w = np.asarray(sr_w, f)
    sr_b = np.asarray(sr_b, f)
    ln_g = np.asarray(ln_g, f)
    ln_b = np.asarray(ln_b, f)
    proj_w = np.asarray(proj_w, f)
    proj_b = np.asarray(proj_b, f)

    import ml_dtypes
    bf = ml_dtypes.bfloat16

    qwT = np.ascontiguousarray(q_w.T * SCALE)
    kwTf = np.ascontiguousarray(ln_g[:, None] * kv_w[:C].T)   # [cin, out]
    vwTf = np.ascontiguousarray(ln_g[:, None] * kv_w[C:].T)
    kfcol_neg = -kwTf.sum(axis=0)    # [C]
    vfcol_neg = -vwTf.sum(axis=0)
    bvec_k = kv_w[:C] @ ln_b         # dropped: constant along m, cancels
    bvec_v = kv_w[C:] @ ln_b
    pbias = proj_b + proj_w @ bvec_v
    pwT = np.ascontiguousarray(proj_w.T)
    w2 = np.ascontiguousarray(sr_w.transpose(2, 3, 1, 0))
    rowd = np.stack([kfcol_neg, vfcol_neg], axis=0)           # [2, C]
    cold = np.stack([sr_b[:128], sr_b[128:],
                     pbias[:128], pbias[128:]], axis=1)       # [128, 4]
    blkd = np.zeros((2, 128, 128), bf)
    for grp in range(2):
        for i in range(128):
            h = grp * 4 + i // 32
            src_row = 32 * (h % 4) + h // 4
            blkd[grp, src_row, i] = 1.0

    x1T = [np.ascontiguousarray(x1[b].T).astype(bf) for b in range(B)]
    x2T = [np.ascontiguousarray(x2[b].T).astype(bf) for b in range(B)]

    in_maps = []
    for core in range(8):
        b, chk = divmod(core, 4)
        in_maps.append({
            "x1t": x1T[b], "x2t": x2T[b],
            "xqt": np.ascontiguousarray(x1T[b][:, chk * NCH:(chk + 1) * NCH]),
            "w2": w2.astype(bf),
            "wall": np.ascontiguousarray(
                np.concatenate([qwT, kwTf, vwTf, pwT], axis=1)).astype(bf),
            "rowd": rowd.astype(bf), "cold": cold.astype(np.float32),
            "blkd": blkd, "eyed": np.eye(8, dtype=np.float32),
        })

    nc = _get_program()
    res = run_bass_kernel_spmd(nc, in_maps, core_ids=list(range(8)))
    _prog_cache["last_result"] = res
    out = np.empty((B, N, C), f)
    for core in range(8):
        b, chk = divmod(core, 4)
        out[b, chk * NCH:(chk + 1) * NCH, :] = res.results[core]["outt"].T
    return out


# revision 24
# speedup vs baseline: 1.1718x; 1.1718x over previous
"""CrossTemporalAttention2 Trainium2 kernel (v2).

Sharding: 8 cores = 2 batches x 4 query-chunks of 1024 rows. Each core runs
the full conv+LN+KV pipeline for its batch (duplicated within the batch
group) and attention + proj for its 1024 query rows.

v2 restructuring vs v1:
- LN folded algebraically: mean subtraction becomes a rank-1 matmul into the
  k/v projection PSUM accumulation; rstd becomes a per-partition scale
  applied inside exp (scores rows are m) / the v1 PSUM->SBUF copy (v1 rows
  are m).  Per-n score offsets cancel in softmax and are dropped; the v-side
  LN bias folds into the proj bias on the host.
- exp is split across engines: ACT (exact, with per-partition scale),
  DVE (4-op cubic minimax poly on [-0.95, 0.95]), GPSIMD (4-op cubic from a
  DVE-staged prescaled bf16 copy).  Scores live in [-0.71, 0.68].
- Emission order starts ACT exp as early as possible: q -> conv(x2) -> k2 ->
  all nh2=0 scores+exp -> conv(x1)+v1 -> nh2=0 U/pden -> normalize ->
  nh2=1 pipelined loop.
- PSUM budget (8 banks): cv[128,512]x2, scA[128,1024]x2, U0, U1; pden rides
  the cv ring after all phase-1 allocations.
"""

import numpy as np

B, N, C = 2, 4096, 256
H, Dh = 8, 32
M = 1024          # (64/2) * (64/2)
NCH = 1024        # query rows per core
SCALE = Dh ** -0.5
EPS = 1e-5

# minimax cubic fit of exp on [-0.95, 0.95] (rel err <= 4.1e-3 fp32)
C3, C2, C1, C0 = 0.15927659, 0.53526688, 1.00884709, 0.99703789

_prog_cache = {}


def _exp_engine(t):
    # t in [0, 64): tile index in emission order. Returns 'act'|'dve'.
    if t % 6 == 1:
        return "dve"
    return "act"


def _build_program():
    import concourse.bass as bass
    import concourse.bacc as bacc
    import concourse.tile as tile
    from concourse import mybir

    f32 = mybir.dt.float32
    bf16 = mybir.dt.bfloat16
    AF = mybir.ActivationFunctionType
    OP = mybir.AluOpType

    nc = bacc.Bacc()

    x1t = nc.dram_tensor("x1t", [C, N], bf16, kind="ExternalInput")
    x2t = nc.dram_tensor("x2t", [C, N], bf16, kind="ExternalInput")
    xqt = nc.dram_tensor("xqt", [C, NCH], bf16, kind="ExternalInput")
    w2d = nc.dram_tensor("w2", [2, 2, C, C], bf16, kind="ExternalInput")
    wall = nc.dram_tensor("wall", [C, 4 * C], bf16, kind="ExternalInput")
    rowd = nc.dram_tensor("rowd", [2, C], bf16, kind="ExternalInput")   # -kfcol, -vfcol
    cold = nc.dram_tensor("cold", [128, 4], f32, kind="ExternalInput")  # srb(2 oh), pb(2 oh)
    blkd = nc.dram_tensor("blkd", [2, 128, 128], bf16, kind="ExternalInput")
    eyed = nc.dram_tensor("eyed", [8, 8], f32, kind="ExternalInput")
    rstd = nc.dram_tensor("rstd", [2, M], f32, kind="Internal")
    outt = nc.dram_tensor("outt", [C, NCH], f32, kind="ExternalOutput")
    import os
    _dbg = os.environ.get("KDBG", "0") == "1"
    if _dbg:
        dbg_mu = nc.dram_tensor("dbg_mu", [2, M], f32, kind="ExternalOutput")
        dbg_rs = nc.dram_tensor("dbg_rs", [2, M], f32, kind="ExternalOutput")
        dbg_rc = nc.dram_tensor("dbg_rc", [2, 128, 8], f32, kind="ExternalOutput")
        dbg_k2 = nc.dram_tensor("dbg_k2", [128, 128], f32, kind="ExternalOutput")
        dbg_v1 = nc.dram_tensor("dbg_v1", [128, 128], f32, kind="ExternalOutput")
        dbg_q = nc.dram_tensor("dbg_q", [128, 128], f32, kind="ExternalOutput")
        dbg_et = nc.dram_tensor("dbg_et", [128, 1024], f32, kind="ExternalOutput")

    with nc.allow_low_precision(reason="bf16 matmul inputs; fp32 PSUM accumulation"), \
         tile.TileContext(nc) as tc:
      with tc.tile_pool(name="pg", bufs=1) as PG, \
           tc.tile_pool(name="psum", bufs=1, space="PSUM") as PS:
        # ================= consts / weights =================
        wallt = [PG.tile([128, 4 * C], bf16, name=f"wall{ch}",
                         tag=f"wall{ch}") for ch in range(2)]
        nc.gpsimd.dma_start(out=wallt[0], in_=wall[0:128, :])
        nc.sync.dma_start(out=wallt[1], in_=wall[128:256, :])
        wsb = {}
        for wi, nm in enumerate(("q", "k", "v", "p")):
            for ch in range(2):
                wsb[(nm, ch)] = wallt[ch][:, wi * C:(wi + 1) * C]
        kfneg = PG.tile([1, C], bf16, name="kfneg", tag="kfneg")
        nc.gpsimd.dma_start(out=kfneg, in_=rowd[0:1, :])
        vfneg = PG.tile([1, C], bf16, name="vfneg", tag="vfneg")
        nc.gpsimd.dma_start(out=vfneg, in_=rowd[1:2, :])
        cols = PG.tile([128, 4], f32, name="cols", tag="cols")
        nc.gpsimd.dma_start(out=cols, in_=cold[:])
        blk = []
        for grp in range(2):
            t = PG.tile([128, 128], bf16, name=f"blk{grp}", tag=f"blk{grp}")
            nc.gpsimd.dma_start(out=t, in_=blkd[grp])
            blk.append(t)
        w2 = []
        for ch in range(2):
            t = PG.tile([128, 2, 2, C], bf16, name=f"w2{ch}", tag=f"w2{ch}")
            nc.scalar.dma_start(
                out=t,
                in_=w2d[:, :, ch * 128:(ch + 1) * 128, :].rearrange(
                    "kh kw c o -> c kh kw o"))
            w2.append(t)
        xq = []
        for ch in range(2):
            t = PG.tile([128, NCH], bf16, name=f"xq{ch}", tag=f"xq{ch}")
            nc.scalar.dma_start(out=t, in_=xqt[ch * 128:(ch + 1) * 128, :])
            xq.append(t)
        # x loads split in N-halves so conv can start on the first half;
        # x2 (k2 path) first, spread over queues.
        xT = {}
        for inp, dram in ((1, x2t), (0, x1t)):
            for ch in range(2):
                t = PG.tile([128, N], bf16, name=f"x{inp}{ch}", tag=f"x{inp}{ch}")
                xT[(inp, ch)] = t
        for half in range(2):
            hs = slice(half * 2048, (half + 1) * 2048)
            nc.gpsimd.dma_start(out=xT[(1, 0)][:, hs], in_=x2t[0:128, hs])
            nc.sync.dma_start(out=xT[(1, 1)][:, hs], in_=x2t[128:256, hs])
        for half in range(2):
            hs = slice(half * 2048, (half + 1) * 2048)
            nc.gpsimd.dma_start(out=xT[(0, 0)][:, hs], in_=x1t[0:128, hs])
            nc.sync.dma_start(out=xT[(0, 1)][:, hs], in_=x1t[128:256, hs])
        # small consts after the bulk loads (gpsimd queue is in order)

        ones1 = PG.tile([1, 128], bf16, name="ones1", tag="ones1")
        nc.vector.memset(ones1, 1.0)
        selc = PG.tile([128, 1], bf16, name="selc", tag="selc")
        nc.vector.memset(selc, 1.0)
        sel8 = PG.tile([128, 2, 8], bf16, name="sel8", tag="sel8")
        nc.vector.memset(sel8, 0.0)
        for g in range(2):
            nc.vector.memset(sel8[:, g, g:g + 1], 1.0)
        eye8 = PG.tile([8, 8], f32, name="eye8", tag="eye8")
        nc.gpsimd.dma_start(out=eye8, in_=eyed[:])
        epsT = PG.tile([1, 1], f32, name="epsT", tag="epsT")
        nc.vector.memset(epsT, EPS)

        # ================= SBUF data tiles =================
        qT = [PG.tile([128, NCH], bf16, name=f"qT{oh}", tag=f"qT{oh}")
              for oh in range(2)]
        xr = {(inp, oh): PG.tile([128, M], bf16, name=f"xr{inp}{oh}",
                                 tag=f"xr{inp}{oh}")
              for inp in range(2) for oh in range(2)}
        # sq tiles shared between the two inputs (x2 stats finish before
        # conv(x1) writes them again; WAR handled by tile deps)
        sq = {}
        for inp in (1, 0):   # allocation order must match usage order
            for oh in range(2):
                sq[(inp, oh)] = PG.tile([128, M], bf16, name=f"sqt{oh}",
                                        tag=f"sqt{oh}", bufs=1)
        k2x = [PG.tile([128, M], bf16, name=f"k2x{oh}", tag=f"k2x{oh}")
               for oh in range(2)]
        v1 = [PG.tile([128, C], bf16, name=f"v1_{ms}", tag=f"v1_{ms}")
              for ms in range(8)]
        murow = [PG.tile([1, M], bf16, name=f"mu{inp}", tag=f"mu{inp}")
                 for inp in range(2)]
        varrow = [PG.tile([1, M], f32, name=f"va{inp}", tag=f"va{inp}")
                  for inp in range(2)]
        rcol = [PG.tile([128, 8], f32, name=f"rcol{inp}", tag=f"rcol{inp}")
                for inp in range(2)]

        def cvt():
            return PS.tile([128, 512], f32, name="cv", tag="cv", bufs=2)

        # ================= q =================
        for oh in range(2):
            for nh in range(2):
                ps = cvt()
                for ch in range(2):
                    nc.tensor.matmul(
                        ps, wsb[("q", ch)][:, oh * 128:(oh + 1) * 128],
                        xq[ch][:, nh * 512:(nh + 1) * 512],
                        start=(ch == 0), stop=(ch == 1))
                nc.scalar.copy(out=qT[oh][:, nh * 512:(nh + 1) * 512], in_=ps)

        # ================= conv + stats + proj-k/v helpers ============
        def conv_half(inp, mh):
            # conv for m-columns [mh*512, (mh+1)*512) of both oh chunks
            for oh in range(2):
                ps = cvt()
                k = 0
                for ch in range(2):
                    xv = xT[(inp, ch)].rearrange(
                        "p (i ki j kj) -> p ki kj i j", ki=2, kj=2, j=32)
                    for kh in range(2):
                        for kw in range(2):
                            nc.tensor.matmul(
                                ps,
                                w2[ch][:, kh, kw, oh * 128:(oh + 1) * 128],
                                xv[:, kh, kw, mh * 16:(mh + 1) * 16, :],
                                start=(k == 0), stop=(k == 7))
                            k += 1
                sl = slice(mh * 512, (mh + 1) * 512)
                # copy + per-partition conv bias (srb col 0/1)
                nc.vector.tensor_scalar(
                    out=xr[(inp, oh)][:, sl], in0=ps,
                    scalar1=cols[:, oh:oh + 1], scalar2=None, op0=OP.add)
                nc.gpsimd.tensor_mul(sq[(inp, oh)][:, sl],
                                     xr[(inp, oh)][:, sl],
                                     xr[(inp, oh)][:, sl])

        def stats_half(inp, mh):
            sl = slice(mh * 512, (mh + 1) * 512)
            pmu = PS.tile([1, 512], f32, name="pmu", tag="scA", bufs=2)
            psq = PS.tile([1, 512], f32, name="psq", tag="scA", bufs=2)
            for k, oh in enumerate(range(2)):
                nc.tensor.matmul(pmu, selc, xr[(inp, oh)][:, sl],
                                 start=(k == 0), stop=(k == 1))
                nc.tensor.matmul(psq, selc, sq[(inp, oh)][:, sl],
                                 start=(k == 0), stop=(k == 1))
            nc.scalar.mul(out=murow[inp][:, sl], in_=pmu, mul=1.0 / C)
            mu2 = PG.tile([1, 512], f32, name="mu2", tag="mu2", bufs=2)
            nc.vector.tensor_mul(mu2, murow[inp][:, sl], murow[inp][:, sl])
            nc.vector.scalar_tensor_tensor(
                out=varrow[inp][:, sl], in0=psq, scalar=1.0 / C, in1=mu2,
                op0=OP.mult, op1=OP.subtract)

        def rcol_make(inp):
            # var row [1, 1024] -> [128, 8] columns (DRAM bounce), then
            # rsqrt(var + eps) on DVE: bit-trick seed + 2 Newton steps.
            nc.gpsimd.dma_start(out=rstd[inp:inp + 1, :], in_=varrow[inp])
            vc = PG.tile([128, 8], f32, name="vc", tag="vc", bufs=2)
            nc.gpsimd.dma_start(
                out=vc,
                in_=rstd[inp:inp + 1, :].rearrange("o (j p) -> (o p) j",
                                                   p=128))
            nc.vector.tensor_scalar(out=vc, in0=vc, scalar1=float(EPS),
                                    scalar2=None, op0=OP.add)
            i32 = mybir.dt.int32
            sh = PG.tile([128, 8], i32, name="sh", tag="sh", bufs=2)
            nc.vector.tensor_scalar(out=sh, in0=vc.bitcast(i32), scalar1=1,
                                    scalar2=None, op0=OP.arith_shift_right)
            y0 = PG.tile([128, 8], i32, name="y0", tag="y0", bufs=2)
            nc.vector.tensor_scalar(out=y0, in0=sh, scalar1=-1,
                                    scalar2=0x5F3759DF,
                                    op0=OP.mult, op1=OP.add)
            y = y0.bitcast(f32)
            for it in range(2):
                c = PG.tile([128, 8], f32, name="nc1", tag="nc1", bufs=2)
                nc.vector.tensor_mul(c, y, y)
                nc.vector.tensor_mul(c, c, vc)
                nc.vector.tensor_scalar(out=c, in0=c, scalar1=-0.5,
                                        scalar2=1.5, op0=OP.mult, op1=OP.add)
                dst = rcol[inp] if it == 1 else y
                nc.vector.tensor_mul(dst, y, c)

        def k2x_half(mh):
            sl = slice(mh * 512, (mh + 1) * 512)
            for oh in range(2):
                ps = cvt()
                for ch in range(2):
                    nc.tensor.matmul(
                        ps, wsb[("k", ch)][:, oh * 128:(oh + 1) * 128],
                        xr[(1, ch)][:, sl], start=(ch == 0), stop=False)
                # rank-1: += (-kfcol) x mu2  (mean subtraction folded in)
                nc.tensor.matmul(
                    ps, kfneg[:, oh * 128:(oh + 1) * 128],
                    murow[1][:, sl], start=False, stop=True)
                nc.vector.tensor_copy(out=k2x[oh][:, sl], in_=ps)

        def v1_chunk(ms):
            msl = slice(ms * 128, (ms + 1) * 128)
            ps = PS.tile([128, C], f32, name="vp", tag="cv", bufs=2)
            for ch in range(2):
                nc.tensor.matmul(ps, xr[(0, ch)][:, msl], wsb[("v", ch)],
                                 start=(ch == 0), stop=False)
            # rank-1: += mu1 x (-vfcol)
            nc.tensor.matmul(ps, murow[0][:, msl], vfneg,
                             start=False, stop=True)
            # copy with per-partition rstd scale folded in
            nc.vector.tensor_scalar(
                out=v1[ms], in0=ps, scalar1=rcol[0][:, ms:ms + 1],
                scalar2=None, op0=OP.mult)

        # ================= phase 2 machinery =================
        ET = tc.alloc_tile_pool(name="et", bufs=(28 if _dbg else 34))
        XG = tc.alloc_tile_pool(name="xg", bufs=3)
        tile_ctr = [0]

        def scores_exp(nh2, ms, grp):
            """scores for 2 pr-tiles (4 heads) + exp on assigned engine.
            Returns the two et tiles."""
            nsl = slice(nh2 * 512, (nh2 + 1) * 512)
            ets = []
            for pr in range(2):
                t = tile_ctr[0]
                tile_ctr[0] += 1
                eng = _exp_engine(t)
                scps = PS.tile([128, 1024], f32, name="scps", tag="scA",
                               bufs=2)
                for i in range(2):
                    h = grp * 4 + pr * 2 + i
                    hb = 32 * (h % 4)
                    nc.tensor.matmul(
                        scps[:, i * 512:(i + 1) * 512],
                        k2x[h // 4][hb:hb + 32, ms * 128:(ms + 1) * 128],
                        qT[h // 4][hb:hb + 32, nsl],
                        start=True, stop=True,
                        tile_position=(hb, 0))
                et = ET.tile([128, 1024], bf16, name="et", tag="et")
                rsc = rcol[1][:, ms:ms + 1]
                if eng == "act":
                    nc.scalar.activation(out=et, in_=scps, func=AF.Exp,
                                         scale=rsc)
                else:  # dve cubic
                    t = XG.tile([128, 1024], bf16, name="t", tag="t", bufs=1)
                    nc.vector.tensor_scalar(
                        out=t, in0=scps, scalar1=rsc, scalar2=None,
                        op0=OP.mult)
                    u = XG.tile([128, 1024], bf16, name="u", tag="u", bufs=1)
                    nc.vector.tensor_mul(u, t, t)
                    v = XG.tile([128, 1024], bf16, name="v", tag="v", bufs=1)
                    nc.vector.tensor_scalar(
                        out=v, in0=t, scalar1=float(C3), scalar2=float(C2),
                        op0=OP.mult, op1=OP.add)
                    p = XG.tile([128, 1024], bf16, name="p", tag="p", bufs=1)
                    nc.vector.tensor_mul(p, u, v)
                    w = XG.tile([128, 1024], bf16, name="w", tag="w", bufs=1)
                    nc.vector.tensor_scalar(
                        out=w, in0=t, scalar1=float(C1), scalar2=float(C0),
                        op0=OP.mult, op1=OP.add)
                    nc.vector.tensor_add(et, w, p)
                ets.append(et)
            return ets

        U = {}
        pden = {}

        def upden(nh2, ms, grp, ets):
            if ms == 0 and grp == 0:
                U[(nh2, 0)] = PS.tile([128, 512], f32, name="U0", tag="U0")
                U[(nh2, 1)] = PS.tile([128, 512], f32, name="U1", tag="U1")
                pden[nh2] = PS.tile([128, 512], f32, name="pden", tag="cv",
                                    bufs=2)
            for pr in range(2):
                for i in range(2):
                    h = grp * 4 + pr * 2 + i
                    h4 = pr * 2 + i
                    esl = ets[pr][:, i * 512:(i + 1) * 512]
                    nc.tensor.matmul(
                        U[(nh2, grp)][32 * h4:32 * h4 + 32, :],
                        v1[ms][:, 32 * h:32 * h + 32], esl,
                        start=(ms == 0), stop=(ms == 7),
                        tile_position=(0, 32 * h4),
                        skip_group_check=True)
            for pr in range(2):
                for i in range(2):
                    h = grp * 4 + pr * 2 + i
                    g = h % 4
                    esl = ets[pr][:, i * 512:(i + 1) * 512]
                    nc.tensor.matmul(
                        pden[nh2][32 * g:32 * g + 8, :],
                        sel8[:, h // 4, :], esl,
                        start=(ms == 0 and grp == 0),
                        stop=(ms == 7 and grp == 1),
                        tile_position=(0, 32 * g),
                        skip_group_check=True)

        def normalize_proj(nh2):
            nsl = slice(nh2 * 512, (nh2 + 1) * 512)
            pdenS = PG.tile([128, 512], bf16, name="pdenS", tag="pdenS",
                            bufs=2)
            nc.vector.tensor_copy(out=pdenS, in_=pden[nh2])
            rps = PS.tile([128, 1024], f32, name="rps", tag="scA", bufs=2)
            for grp in range(2):
                nc.tensor.matmul(rps[:, grp * 512:(grp + 1) * 512],
                                 blk[grp], pdenS, start=True, stop=True)
            recf = PG.tile([128, 1024], f32, name="recf", tag="recf",
                           bufs=2)
            nc.vector.reciprocal_approx_fast(out=recf, in_=rps)
            oT = []
            for grp in range(2):
                ot = PG.tile([128, 512], bf16, name="ot", tag=f"ot{grp}",
                             bufs=2)
                nc.vector.tensor_mul(ot, U[(nh2, grp)],
                                     recf[:, grp * 512:(grp + 1) * 512])
                oT.append(ot)
            for oh in range(2):
                ps = cvt()
                for ch in range(2):
                    nc.tensor.matmul(
                        ps, wsb[("p", ch)][:, oh * 128:(oh + 1) * 128],
                        oT[ch], start=(ch == 0), stop=(ch == 1))
                y = PG.tile([128, 512], f32, name="y", tag="y", bufs=2)
                # copy + per-partition proj bias (pb col 2/3)
                nc.vector.tensor_scalar(
                    out=y, in0=ps, scalar1=cols[:, 2 + oh:3 + oh],
                    scalar2=None, op0=OP.add)
                nc.gpsimd.dma_start(out=outt[oh * 128:(oh + 1) * 128, nsl],
                                    in_=y)

        # ================= emission =================
        # phase 1a: conv(x2) + stats + k2x  (rcol as soon as stats done)
        conv_half(1, 0)
        stats_half(1, 0)
        conv_half(1, 1)
        stats_half(1, 1)
        rcol_make(1)
        k2x_half(0)
        k2x_half(1)

        # phase 2a: nh2=0 scores + exp, with phase 1b (conv(x1)+v1) steps
        # interleaved so the PE stays dense while ACT chews on exp.
        p1b_steps = (
            [lambda: conv_half(0, 0)] +
            [lambda: stats_half(0, 0)] +
            [lambda: conv_half(0, 1)] +
            [lambda: stats_half(0, 1), lambda: rcol_make(0)] +
            [lambda ms=ms: v1_chunk(ms) for ms in range(8)] +
            [lambda: None] * 3
        )
        ets0 = {}
        step = 0
        for ms in range(8):
            for grp in range(2):
                ets0[(ms, grp)] = scores_exp(0, ms, grp)
                p1b_steps[step]()
                step += 1

        if _dbg:
            for inp in range(2):
                mcf = PG.tile([1, M], f32, name="mcf", tag="mcf", bufs=1)
                nc.vector.tensor_copy(out=mcf, in_=murow[inp])
                nc.gpsimd.dma_start(out=dbg_mu[inp:inp+1, :], in_=mcf)
                nc.gpsimd.dma_start(out=dbg_rs[inp:inp+1, :], in_=rsrow[inp])
                nc.gpsimd.dma_start(out=dbg_rc[inp], in_=rcol[inp])
            smp = PG.tile([128, 128], f32, name="smp", tag="smp", bufs=2)
            for nmd, srcd in (("dbg_k2", k2x[0]), ("dbg_v1", v1[0]),
                              ("dbg_q", qT[0])):
                s = PG.tile([128, 128], f32, name="s_" + nmd, tag="smp",
                            bufs=2)
                nc.vector.tensor_copy(out=s, in_=srcd[:, 0:128])
                d = {"dbg_k2": dbg_k2, "dbg_v1": dbg_v1, "dbg_q": dbg_q}[nmd]
                nc.gpsimd.dma_start(out=d[:, 0:128], in_=s)
            etf = PG.tile([128, 1024], f32, name="etf", tag="etf")
            nc.vector.tensor_copy(out=etf, in_=ets0[(0, 0)][0])
            nc.gpsimd.dma_start(out=dbg_et[:], in_=etf)

        # phase 2b: nh2=0 U/pden
        for ms in range(8):
            for grp in range(2):
                upden(0, ms, grp, ets0[(ms, grp)])
        normalize_proj(0)

        # phase 2c: nh2=1, software pipelined (U/pden one iteration behind)
        prev = None
        for ms in range(8):
            for grp in range(2):
                ets = scores_exp(1, ms, grp)
                if prev is not None:
                    upden(1, *prev)
                prev = (ms, grp, ets)
        upden(1, *prev)
        normalize_proj(1)

        XG.release()
        ET.release()
    nc.finalize()
    return nc


def _get_program():
    if "nc" not in _prog_cache:
        _prog_cache["nc"] = _build_program()
    return _prog_cache["nc"]


def kernel(x1, x2, q_w, kv_w, sr_w, sr_b, ln_g, ln_b, proj_w, proj_b,
           H1=64, W1=64, H2=64, W2=64, **_):
    from concourse.bass_utils import run_bass_kernel_spmd

    f = np.float32
    x1 = np.asarray(x1, f)
    x2 = np.asarray(x2, f)
    q_w = np.asarray(q_w, f)
    kv_w = np.asarray(kv_w, f)
    sr_w = np.asarray(sr_w, f)
    sr_b = np.asarray(sr_b, f)
    ln_g = np.asarray(ln_g, f)
    ln_b = np.asarray(ln_b, f)
    proj_w = np.asarray(proj_w, f)
    proj_b = np.asarray(proj_b, f)

    import ml_dtypes
    bf = ml_dtypes.bfloat16

    qwT = np.ascontiguousarray(q_w.T * SCALE)
    kwTf = np.ascontiguousarray(ln_g[:, None] * kv_w[:C].T)   # [cin, out]
    vwTf = np.ascontiguousarray(ln_g[:, None] * kv_w[C:].T)
    kfcol_neg = -kwTf.sum(axis=0)    # [C]
    vfcol_neg = -vwTf.sum(axis=0)
    bvec_k = kv_w[:C] @ ln_b         # dropped: constant along m, cancels
    bvec_v = kv_w[C:] @ ln_b
    pbias = proj_b + proj_w @ bvec_v
    pwT = np.ascontiguousarray(proj_w.T)
    w2 = np.ascontiguousarray(sr_w.transpose(2, 3, 1, 0))
    rowd = np.stack([kfcol_neg, vfcol_neg], axis=0)           # [2, C]
    cold = np.stack([sr_b[:128], sr_b[128:],
                     pbias[:128], pbias[128:]], axis=1)       # [128, 4]
    blkd = np.zeros((2, 128, 128), bf)
    for grp in range(2):
        for i in range(128):
            h = grp * 4 + i // 32
            src_row = 32 * (h % 4) + h // 4
            blkd[grp, src_row, i] = 1.0

    x1T = [np.ascontiguousarray(x1[b].T).astype(bf) for b in range(B)]
    x2T = [np.ascontiguousarray(x2[b].T).astype(bf) for b in range(B)]

    in_maps = []
    for core in range(8):
        b, chk = divmod(core, 4)
        in_maps.append({
            "x1t": x1T[b], "x2t": x2T[b],
            "xqt": np.ascontiguousarray(x1T[b][:, chk * NCH:(chk + 1) * NCH]),
            "w2": w2.astype(bf),
            "wall": np.ascontiguousarray(
                np.concatenate([qwT, kwTf, vwTf, pwT], axis=1)).astype(bf),
            "rowd": rowd.astype(bf), "cold": cold.astype(np.float32),
            "blkd": blkd, "eyed": np.eye(8, dtype=np.float32),
        })

    nc = _get_program()
    res = run_bass_kernel_spmd(nc, in_maps, core_ids=list(range(8)))
    _prog_cache["last_result"] = res
    out = np.empty((B, N, C), f)
    for core in range(8):
        b, chk = divmod(core, 4)
        out[b, chk * NCH:(chk + 1) * NCH, :] = res.results[core]["outt"].T
    return out


# revision 25
# speedup vs baseline: 1.2027x; 1.0264x over previous
"""CrossTemporalAttention2 Trainium2 kernel (v2).

Sharding: 8 cores = 2 batches x 4 query-chunks of 1024 rows. Each core runs
the full conv+LN+KV pipeline for its batch (duplicated within the batch
group) and attention + proj for its 1024 query rows.

v2 restructuring vs v1:
- LN folded algebraically: mean subtraction becomes a rank-1 matmul into the
  k/v projection PSUM accumulation; rstd becomes a per-partition scale
  applied inside exp (scores rows are m) / the v1 PSUM->SBUF copy (v1 rows
  are m).  Per-n score offsets cancel in softmax and are dropped; the v-side
  LN bias folds into the proj bias on the host.
- exp is split across engines: ACT (exact, with per-partition scale),
  DVE (4-op cubic minimax poly on [-0.95, 0.95]), GPSIMD (4-op cubic from a
  DVE-staged prescaled bf16 copy).  Scores live in [-0.71, 0.68].
- Emission order starts ACT exp as early as possible: q -> conv(x2) -> k2 ->
  all nh2=0 scores+exp -> conv(x1)+v1 -> nh2=0 U/pden -> normalize ->
  nh2=1 pipelined loop.
- PSUM budget (8 banks): cv[128,512]x2, scA[128,1024]x2, U0, U1; pden rides
  the cv ring after all phase-1 allocations.
"""

import numpy as np

B, N, C = 2, 4096, 256
H, Dh = 8, 32
M = 1024          # (64/2) * (64/2)
NCH = 1024        # query rows per core
SCALE = Dh ** -0.5
EPS = 1e-5

# minimax cubic fit of exp on [-0.95, 0.95] (rel err <= 4.1e-3 fp32)
C3, C2, C1, C0 = 0.15927659, 0.53526688, 1.00884709, 0.99703789

_prog_cache = {}


def _exp_engine(t):
    # t in [0, 64): tile index in emission order. Returns 'act'|'dve'.
    if t % 6 == 1:
        return "dve"
    return "act"


def _build_program():
    import concourse.bass as bass
    import concourse.bacc as bacc
    import concourse.tile as tile
    from concourse import mybir

    f32 = mybir.dt.float32
    bf16 = mybir.dt.bfloat16
    AF = mybir.ActivationFunctionType
    OP = mybir.AluOpType

    nc = bacc.Bacc()

    x1t = nc.dram_tensor("x1t", [C, N], bf16, kind="ExternalInput")
    x2t = nc.dram_tensor("x2t", [C, N], bf16, kind="ExternalInput")
    xqt = nc.dram_tensor("xqt", [C, NCH], bf16, kind="ExternalInput")
    w2d = nc.dram_tensor("w2", [2, 2, C, C], bf16, kind="ExternalInput")
    wall = nc.dram_tensor("wall", [C, 4 * C], bf16, kind="ExternalInput")
    rowd = nc.dram_tensor("rowd", [2, C], bf16, kind="ExternalInput")   # -kfcol, -vfcol
    cold = nc.dram_tensor("cold", [128, 4], f32, kind="ExternalInput")  # srb(2 oh), pb(2 oh)
    blkd = nc.dram_tensor("blkd", [2, 128, 128], bf16, kind="ExternalInput")
    eyed = nc.dram_tensor("eyed", [8, 8], f32, kind="ExternalInput")
    rstd = nc.dram_tensor("rstd", [2, M], f32, kind="Internal")
    outt = nc.dram_tensor("outt", [C, NCH], f32, kind="ExternalOutput")
    import os
    _dbg = os.environ.get("KDBG", "0") == "1"
    if _dbg:
        dbg_mu = nc.dram_tensor("dbg_mu", [2, M], f32, kind="ExternalOutput")
        dbg_rs = nc.dram_tensor("dbg_rs", [2, M], f32, kind="ExternalOutput")
        dbg_rc = nc.dram_tensor("dbg_rc", [2, 128, 8], f32, kind="ExternalOutput")
        dbg_k2 = nc.dram_tensor("dbg_k2", [128, 128], f32, kind="ExternalOutput")
        dbg_v1 = nc.dram_tensor("dbg_v1", [128, 128], f32, kind="ExternalOutput")
        dbg_q = nc.dram_tensor("dbg_q", [128, 128], f32, kind="ExternalOutput")
        dbg_et = nc.dram_tensor("dbg_et", [128, 1024], f32, kind="ExternalOutput")

    with nc.allow_low_precision(reason="bf16 matmul inputs; fp32 PSUM accumulation"), \
         tile.TileContext(nc) as tc:
      with tc.tile_pool(name="pg", bufs=1) as PG, \
           tc.tile_pool(name="psum", bufs=1, space="PSUM") as PS:
        # ================= consts / weights =================
        wallt = [PG.tile([128, 4 * C], bf16, name=f"wall{ch}",
                         tag=f"wall{ch}") for ch in range(2)]
        nc.gpsimd.dma_start(out=wallt[0], in_=wall[0:128, :])
        nc.sync.dma_start(out=wallt[1], in_=wall[128:256, :])
        wsb = {}
        for wi, nm in enumerate(("q", "k", "v", "p")):
            for ch in range(2):
                wsb[(nm, ch)] = wallt[ch][:, wi * C:(wi + 1) * C]
        kfneg = PG.tile([1, C], bf16, name="kfneg", tag="kfneg")
        nc.gpsimd.dma_start(out=kfneg, in_=rowd[0:1, :])
        vfneg = PG.tile([1, C], bf16, name="vfneg", tag="vfneg")
        nc.gpsimd.dma_start(out=vfneg, in_=rowd[1:2, :])
        cols = PG.tile([128, 4], f32, name="cols", tag="cols")
        nc.gpsimd.dma_start(out=cols, in_=cold[:])
        blk = []
        for grp in range(2):
            t = PG.tile([128, 128], bf16, name=f"blk{grp}", tag=f"blk{grp}")
            nc.gpsimd.dma_start(out=t, in_=blkd[grp])
            blk.append(t)
        w2 = []
        for ch in range(2):
            t = PG.tile([128, 2, 2, C], bf16, name=f"w2{ch}", tag=f"w2{ch}")
            nc.scalar.dma_start(
                out=t,
                in_=w2d[:, :, ch * 128:(ch + 1) * 128, :].rearrange(
                    "kh kw c o -> c kh kw o"))
            w2.append(t)
        xq = []
        for ch in range(2):
            t = PG.tile([128, NCH], bf16, name=f"xq{ch}", tag=f"xq{ch}")
            nc.scalar.dma_start(out=t, in_=xqt[ch * 128:(ch + 1) * 128, :])
            xq.append(t)
        # x loads split in N-halves so conv can start on the first half;
        # x2 (k2 path) first, spread over queues.
        xT = {}
        for inp, dram in ((1, x2t), (0, x1t)):
            for ch in range(2):
                t = PG.tile([128, N], bf16, name=f"x{inp}{ch}", tag=f"x{inp}{ch}")
                xT[(inp, ch)] = t
        for half in range(2):
            hs = slice(half * 2048, (half + 1) * 2048)
            nc.gpsimd.dma_start(out=xT[(1, 0)][:, hs], in_=x2t[0:128, hs])
            nc.sync.dma_start(out=xT[(1, 1)][:, hs], in_=x2t[128:256, hs])
        for half in range(2):
            hs = slice(half * 2048, (half + 1) * 2048)
            nc.gpsimd.dma_start(out=xT[(0, 0)][:, hs], in_=x1t[0:128, hs])
            nc.sync.dma_start(out=xT[(0, 1)][:, hs], in_=x1t[128:256, hs])
        # small consts after the bulk loads (gpsimd queue is in order)

        ones1 = PG.tile([1, 128], bf16, name="ones1", tag="ones1")
        nc.vector.memset(ones1, 1.0)
        selc = PG.tile([128, 1], bf16, name="selc", tag="selc")
        nc.vector.memset(selc, 1.0)
        sel8 = PG.tile([128, 2, 8], bf16, name="sel8", tag="sel8")
        nc.vector.memset(sel8, 0.0)
        for g in range(2):
            nc.vector.memset(sel8[:, g, g:g + 1], 1.0)
        eye8 = PG.tile([8, 8], f32, name="eye8", tag="eye8")
        nc.gpsimd.dma_start(out=eye8, in_=eyed[:])
        epsT = PG.tile([1, 1], f32, name="epsT", tag="epsT")
        nc.vector.memset(epsT, EPS)

        # ================= SBUF data tiles =================
        qT = [PG.tile([128, NCH], bf16, name=f"qT{oh}", tag=f"qT{oh}")
              for oh in range(2)]
        xr = {(inp, oh): PG.tile([128, M], bf16, name=f"xr{inp}{oh}",
                                 tag=f"xr{inp}{oh}")
              for inp in range(2) for oh in range(2)}
        # sq tiles shared between the two inputs (x2 stats finish before
        # conv(x1) writes them again; WAR handled by tile deps)
        sq = {}
        for inp in (1, 0):   # allocation order must match usage order
            for oh in range(2):
                sq[(inp, oh)] = PG.tile([128, M], bf16, name=f"sqt{oh}",
                                        tag=f"sqt{oh}", bufs=1)
        k2x = [PG.tile([128, M], bf16, name=f"k2x{oh}", tag=f"k2x{oh}")
               for oh in range(2)]
        v1 = [PG.tile([128, C], bf16, name=f"v1_{ms}", tag=f"v1_{ms}")
              for ms in range(8)]
        murow = [PG.tile([1, M], bf16, name=f"mu{inp}", tag=f"mu{inp}")
                 for inp in range(2)]
        varrow = [PG.tile([1, M], f32, name=f"va{inp}", tag=f"va{inp}")
                  for inp in range(2)]
        rcol = [PG.tile([128, 8], f32, name=f"rcol{inp}", tag=f"rcol{inp}")
                for inp in range(2)]

        def cvt():
            return PS.tile([128, 512], f32, name="cv", tag="cv", bufs=2)

        # ================= q =================
        for oh in range(2):
            for nh in range(2):
                ps = cvt()
                for ch in range(2):
                    nc.tensor.matmul(
                        ps, wsb[("q", ch)][:, oh * 128:(oh + 1) * 128],
                        xq[ch][:, nh * 512:(nh + 1) * 512],
                        start=(ch == 0), stop=(ch == 1))
                nc.scalar.copy(out=qT[oh][:, nh * 512:(nh + 1) * 512], in_=ps)

        # ================= conv + stats + proj-k/v helpers ============
        def conv_half(inp, mh):
            # conv for m-columns [mh*512, (mh+1)*512) of both oh chunks
            for oh in range(2):
                ps = cvt()
                k = 0
                for ch in range(2):
                    xv = xT[(inp, ch)].rearrange(
                        "p (i ki j kj) -> p ki kj i j", ki=2, kj=2, j=32)
                    for kh in range(2):
                        for kw in range(2):
                            nc.tensor.matmul(
                                ps,
                                w2[ch][:, kh, kw, oh * 128:(oh + 1) * 128],
                                xv[:, kh, kw, mh * 16:(mh + 1) * 16, :],
                                start=(k == 0), stop=(k == 7))
                            k += 1
                sl = slice(mh * 512, (mh + 1) * 512)
                # copy + per-partition conv bias (srb col 0/1)
                nc.vector.tensor_scalar(
                    out=xr[(inp, oh)][:, sl], in0=ps,
                    scalar1=cols[:, oh:oh + 1], scalar2=None, op0=OP.add)
                nc.gpsimd.tensor_mul(sq[(inp, oh)][:, sl],
                                     xr[(inp, oh)][:, sl],
                                     xr[(inp, oh)][:, sl])

        def stats_half(inp, mh):
            sl = slice(mh * 512, (mh + 1) * 512)
            pmu = PS.tile([1, 512], f32, name="pmu", tag="scA", bufs=2)
            psq = PS.tile([1, 512], f32, name="psq", tag="scA", bufs=2)
            for k, oh in enumerate(range(2)):
                nc.tensor.matmul(pmu, selc, xr[(inp, oh)][:, sl],
                                 start=(k == 0), stop=(k == 1))
                nc.tensor.matmul(psq, selc, sq[(inp, oh)][:, sl],
                                 start=(k == 0), stop=(k == 1))
            nc.scalar.mul(out=murow[inp][:, sl], in_=pmu, mul=1.0 / C)
            mu2 = PG.tile([1, 512], f32, name="mu2", tag="mu2", bufs=2)
            nc.vector.tensor_mul(mu2, murow[inp][:, sl], murow[inp][:, sl])
            nc.vector.scalar_tensor_tensor(
                out=varrow[inp][:, sl], in0=psq, scalar=1.0 / C, in1=mu2,
                op0=OP.mult, op1=OP.subtract)

        def rcol_make(inp):
            # var row [1, 1024] -> [128, 8] columns (DRAM bounce), then
            # rsqrt(var + eps) on DVE: bit-trick seed + 2 Newton steps.
            nc.gpsimd.dma_start(out=rstd[inp:inp + 1, :], in_=varrow[inp])
            vc = PG.tile([128, 8], f32, name="vc", tag="vc", bufs=2)
            nc.gpsimd.dma_start(
                out=vc,
                in_=rstd[inp:inp + 1, :].rearrange("o (j p) -> (o p) j",
                                                   p=128))
            nc.vector.tensor_scalar(out=vc, in0=vc, scalar1=float(EPS),
                                    scalar2=None, op0=OP.add)
            i32 = mybir.dt.int32
            sh = PG.tile([128, 8], i32, name="sh", tag="sh", bufs=2)
            nc.vector.tensor_scalar(out=sh, in0=vc.bitcast(i32), scalar1=1,
                                    scalar2=None, op0=OP.arith_shift_right)
            y0 = PG.tile([128, 8], i32, name="y0", tag="y0", bufs=2)
            nc.vector.tensor_scalar(out=y0, in0=sh, scalar1=-1,
                                    scalar2=0x5F3759DF,
                                    op0=OP.mult, op1=OP.add)
            y = y0.bitcast(f32)
            for it in range(2):
                c = PG.tile([128, 8], f32, name="nc1", tag="nc1", bufs=2)
                nc.vector.tensor_mul(c, y, y)
                nc.vector.tensor_mul(c, c, vc)
                nc.vector.tensor_scalar(out=c, in0=c, scalar1=-0.5,
                                        scalar2=1.5, op0=OP.mult, op1=OP.add)
                dst = rcol[inp] if it == 1 else y
                nc.vector.tensor_mul(dst, y, c)

        def k2x_half(mh):
            sl = slice(mh * 512, (mh + 1) * 512)
            for oh in range(2):
                ps = cvt()
                for ch in range(2):
                    nc.tensor.matmul(
                        ps, wsb[("k", ch)][:, oh * 128:(oh + 1) * 128],
                        xr[(1, ch)][:, sl], start=(ch == 0), stop=False)
                # rank-1: += (-kfcol) x mu2  (mean subtraction folded in)
                nc.tensor.matmul(
                    ps, kfneg[:, oh * 128:(oh + 1) * 128],
                    murow[1][:, sl], start=False, stop=True)
                nc.vector.tensor_copy(out=k2x[oh][:, sl], in_=ps)

        def v1_chunk(ms):
            msl = slice(ms * 128, (ms + 1) * 128)
            ps = PS.tile([128, C], f32, name="vp", tag="cv", bufs=2)
            for ch in range(2):
                nc.tensor.matmul(ps, xr[(0, ch)][:, msl], wsb[("v", ch)],
                                 start=(ch == 0), stop=False)
            # rank-1: += mu1 x (-vfcol)
            nc.tensor.matmul(ps, murow[0][:, msl], vfneg,
                             start=False, stop=True)
            # copy with per-partition rstd scale folded in
            nc.vector.tensor_scalar(
                out=v1[ms], in0=ps, scalar1=rcol[0][:, ms:ms + 1],
                scalar2=None, op0=OP.mult)

        # ================= phase 2 machinery =================
        ET = tc.alloc_tile_pool(name="et", bufs=(28 if _dbg else 34))
        XG = tc.alloc_tile_pool(name="xg", bufs=3)
        tile_ctr = [0]

        def scores_exp(nh2, ms, grp):
            """scores for 2 pr-tiles (4 heads) + exp on assigned engine.
            Returns the two et tiles."""
            nsl = slice(nh2 * 512, (nh2 + 1) * 512)
            ets = []
            for pr in range(2):
                t = tile_ctr[0]
                tile_ctr[0] += 1
                eng = _exp_engine(t)
                scps = PS.tile([128, 1024], f32, name="scps", tag="scA",
                               bufs=2)
                for i in range(2):
                    h = grp * 4 + pr * 2 + i
                    hb = 32 * (h % 4)
                    nc.tensor.matmul(
                        scps[:, i * 512:(i + 1) * 512],
                        k2x[h // 4][hb:hb + 32, ms * 128:(ms + 1) * 128],
                        qT[h // 4][hb:hb + 32, nsl],
                        start=True, stop=True,
                        tile_position=(hb, 0))
                et = ET.tile([128, 1024], bf16, name="et", tag="et")
                rsc = rcol[1][:, ms:ms + 1]
                if eng == "act":
                    nc.scalar.activation(out=et, in_=scps, func=AF.Exp,
                                         scale=rsc)
                else:  # dve cubic
                    t = XG.tile([128, 1024], bf16, name="t", tag="t", bufs=1)
                    nc.vector.tensor_scalar(
                        out=t, in0=scps, scalar1=rsc, scalar2=None,
                        op0=OP.mult)
                    u = XG.tile([128, 1024], bf16, name="u", tag="u", bufs=1)
                    nc.vector.tensor_mul(u, t, t)
                    v = XG.tile([128, 1024], bf16, name="v", tag="v", bufs=1)
                    nc.vector.tensor_scalar(
                        out=v, in0=t, scalar1=float(C3), scalar2=float(C2),
                        op0=OP.mult, op1=OP.add)
                    p = XG.tile([128, 1024], bf16, name="p", tag="p", bufs=1)
                    nc.vector.tensor_mul(p, u, v)
                    w = XG.tile([128, 1024], bf16, name="w", tag="w", bufs=1)
                    nc.vector.tensor_scalar(
                        out=w, in0=t, scalar1=float(C1), scalar2=float(C0),
                        op0=OP.mult, op1=OP.add)
                    nc.vector.tensor_add(et, w, p)
                ets.append(et)
            return ets

        U = {}
        pden = {}

        def upden(nh2, ms, grp, ets):
            if ms == 0 and grp == 0:
                U[(nh2, 0)] = PS.tile([128, 512], f32, name="U0", tag="U0")
                U[(nh2, 1)] = PS.tile([128, 512], f32, name="U1", tag="U1")
                pden[nh2] = PS.tile([128, 512], f32, name="pden", tag="cv",
                                    bufs=2)
            for pr in range(2):
                for i in range(2):
                    h = grp * 4 + pr * 2 + i
                    h4 = pr * 2 + i
                    esl = ets[pr][:, i * 512:(i + 1) * 512]
                    nc.tensor.matmul(
                        U[(nh2, grp)][32 * h4:32 * h4 + 32, :],
                        v1[ms][:, 32 * h:32 * h + 32], esl,
                        start=(ms == 0), stop=(ms == 7),
                        tile_position=(0, 32 * h4),
                        skip_group_check=True)
            for pr in range(2):
                for i in range(2):
                    h = grp * 4 + pr * 2 + i
                    g = h % 4
                    esl = ets[pr][:, i * 512:(i + 1) * 512]
                    nc.tensor.matmul(
                        pden[nh2][32 * g:32 * g + 8, :],
                        sel8[:, h // 4, :], esl,
                        start=(ms == 0 and grp == 0),
                        stop=(ms == 7 and grp == 1),
                        tile_position=(0, 32 * g),
                        skip_group_check=True)

        def normalize_proj(nh2):
            nsl = slice(nh2 * 512, (nh2 + 1) * 512)
            pdenS = PG.tile([128, 512], bf16, name="pdenS", tag="pdenS",
                            bufs=2)
            nc.vector.tensor_copy(out=pdenS, in_=pden[nh2])
            rps = PS.tile([128, 1024], f32, name="rps", tag="scA", bufs=2)
            for grp in range(2):
                nc.tensor.matmul(rps[:, grp * 512:(grp + 1) * 512],
                                 blk[grp], pdenS, start=True, stop=True)
            recf = PG.tile([128, 1024], f32, name="recf", tag="recf",
                           bufs=2)
            nc.vector.reciprocal_approx_fast(out=recf, in_=rps)
            oT = []
            for grp in range(2):
                ot = PG.tile([128, 512], bf16, name="ot", tag=f"ot{grp}",
                             bufs=2)
                nc.vector.tensor_mul(ot, U[(nh2, grp)],
                                     recf[:, grp * 512:(grp + 1) * 512])
                oT.append(ot)
            for oh in range(2):
                ps = cvt()
                for ch in range(2):
                    nc.tensor.matmul(
                        ps, wsb[("p", ch)][:, oh * 128:(oh + 1) * 128],
                        oT[ch], start=(ch == 0), stop=(ch == 1))
                y = PG.tile([128, 512], f32, name="y", tag="y", bufs=2)
                # copy + per-partition proj bias (pb col 2/3)
                nc.vector.tensor_scalar(
                    out=y, in0=ps, scalar1=cols[:, 2 + oh:3 + oh],
                    scalar2=None, op0=OP.add)
                nc.gpsimd.dma_start(out=outt[oh * 128:(oh + 1) * 128, nsl],
                                    in_=y)

        # ================= emission =================
        # phase 1a: conv(x2) + stats + k2x  (rcol as soon as stats done)
        conv_half(1, 0)
        stats_half(1, 0)
        conv_half(1, 1)
        stats_half(1, 1)
        rcol_make(1)
        k2x_half(0)
        k2x_half(1)

        # phase 2a: nh2=0 scores + exp, with phase 1b (conv(x1)+v1) steps
        # interleaved so the PE stays dense while ACT chews on exp.
        p1b_steps = (
            [lambda: conv_half(0, 0)] +
            [lambda: stats_half(0, 0)] +
            [lambda: conv_half(0, 1)] +
            [lambda: stats_half(0, 1), lambda: rcol_make(0)] +
            [lambda ms=ms: v1_chunk(ms) for ms in range(8)] +
            [lambda: None] * 3
        )
        ets0 = {}
        step = 0
        for ms in range(8):
            for grp in range(2):
                ets0[(ms, grp)] = scores_exp(0, ms, grp)
                p1b_steps[step]()
                step += 1

        if _dbg:
            for inp in range(2):
                mcf = PG.tile([1, M], f32, name="mcf", tag="mcf", bufs=1)
                nc.vector.tensor_copy(out=mcf, in_=murow[inp])
                nc.gpsimd.dma_start(out=dbg_mu[inp:inp+1, :], in_=mcf)
                nc.gpsimd.dma_start(out=dbg_rs[inp:inp+1, :], in_=rsrow[inp])
                nc.gpsimd.dma_start(out=dbg_rc[inp], in_=rcol[inp])
            smp = PG.tile([128, 128], f32, name="smp", tag="smp", bufs=2)
            for nmd, srcd in (("dbg_k2", k2x[0]), ("dbg_v1", v1[0]),
                              ("dbg_q", qT[0])):
                s = PG.tile([128, 128], f32, name="s_" + nmd, tag="smp",
                            bufs=2)
                nc.vector.tensor_copy(out=s, in_=srcd[:, 0:128])
                d = {"dbg_k2": dbg_k2, "dbg_v1": dbg_v1, "dbg_q": dbg_q}[nmd]
                nc.gpsimd.dma_start(out=d[:, 0:128], in_=s)
            etf = PG.tile([128, 1024], f32, name="etf", tag="etf")
            nc.vector.tensor_copy(out=etf, in_=ets0[(0, 0)][0])
            nc.gpsimd.dma_start(out=dbg_et[:], in_=etf)

        # phase 2b: interleave nh2=1 scores+exp with nh2=0 U/pden so the
        # ACT engine never starves while the PE drains the stored et tiles
        ets1 = {}
        for ms in range(8):
            for grp in range(2):
                ets1[(ms, grp)] = scores_exp(1, ms, grp)
                upden(0, ms, grp, ets0[(ms, grp)])
        normalize_proj(0)

        # phase 2c: nh2=1 U/pden
        for ms in range(8):
            for grp in range(2):
                upden(1, ms, grp, ets1[(ms, grp)])
        normalize_proj(1)

        XG.release()
        ET.release()
    nc.finalize()
    return nc


def _get_program():
    if "nc" not in _prog_cache:
        _prog_cache["nc"] = _build_program()
    return _prog_cache["nc"]


def kernel(x1, x2, q_w, kv_w, sr_w, sr_b, ln_g, ln_b, proj_w, proj_b,
           H1=64, W1=64, H2=64, W2=64, **_):
    from concourse.bass_utils import run_bass_kernel_spmd

    f = np.float32
    x1 = np.asarray(x1, f)
    x2 = np.asarray(x2, f)
    q_w = np.asarray(q_w, f)
    kv_w = np.asarray(kv_w, f)
    sr_w = np.asarray(sr_w, f)
    sr_b = np.asarray(sr_b, f)
    ln_g = np.asarray(ln_g, f)
    ln_b = np.asarray(ln_b, f)
    proj_w = np.asarray(proj_w, f)
    proj_b = np.asarray(proj_b, f)

    import ml_dtypes
    bf = ml_dtypes.bfloat16

    qwT = np.ascontiguousarray(q_w.T * SCALE)
    kwTf = np.ascontiguousarray(ln_g[:, None] * kv_w[:C].T)   # [cin, out]
    vwTf = np.ascontiguousarray(ln_g[:, None] * kv_w[C:].T)
    kfcol_neg = -kwTf.sum(axis=0)    # [C]
    vfcol_neg = -vwTf.sum(axis=0)
    bvec_k = kv_w[:C] @ ln_b         # dropped: constant along m, cancels
    bvec_v = kv_w[C:] @ ln_b
    pbias = proj_b + proj_w @ bvec_v
    pwT = np.ascontiguousarray(proj_w.T)
    w2 = np.ascontiguousarray(sr_w.transpose(2, 3, 1, 0))
    rowd = np.stack([kfcol_neg, vfcol_neg], axis=0)           # [2, C]
    cold = np.stack([sr_b[:128], sr_b[128:],
                     pbias[:128], pbias[128:]], axis=1)       # [128, 4]
    blkd = np.zeros((2, 128, 128), bf)
    for grp in range(2):
        for i in range(128):
            h = grp * 4 + i // 32
            src_row = 32 * (h % 4) + h // 4
            blkd[grp, src_row, i] = 1.0

    x1T = [np.ascontiguousarray(x1[b].T).astype(bf) for b in range(B)]
    x2T = [np.ascontiguousarray(x2[b].T).astype(bf) for b in range(B)]

    in_maps = []
    for core in range(8):
        b, chk = divmod(core, 4)
        in_maps.append({
            "x1t": x1T[b], "x2t": x2T[b],
            "xqt": np.ascontiguousarray(x1T[b][:, chk * NCH:(chk + 1) * NCH]),
            "w2": w2.astype(bf),
            "wall": np.ascontiguousarray(
                np.concatenate([qwT, kwTf, vwTf, pwT], axis=1)).astype(bf),
            "rowd": rowd.astype(bf), "cold": cold.astype(np.float32),
            "blkd": blkd, "eyed": np.eye(8, dtype=np.float32),
        })

    nc = _get_program()
    res = run_bass_kernel_spmd(nc, in_maps, core_ids=list(range(8)))
    _prog_cache["last_result"] = res
    out = np.empty((B, N, C), f)
    for core in range(8):
        b, chk = divmod(core, 4)
        out[b, chk * NCH:(chk + 1) * NCH, :] = res.results[core]["outt"].T
    return out


# revision 26
# speedup vs baseline: 1.2075x; 1.0040x over previous
"""CrossTemporalAttention2 Trainium2 kernel (v2).

Sharding: 8 cores = 2 batches x 4 query-chunks of 1024 rows. Each core runs
the full conv+LN+KV pipeline for its batch (duplicated within the batch
group) and attention + proj for its 1024 query rows.

v2 restructuring vs v1:
- LN folded algebraically: mean subtraction becomes a rank-1 matmul into the
  k/v projection PSUM accumulation; rstd becomes a per-partition scale
  applied inside exp (scores rows are m) / the v1 PSUM->SBUF copy (v1 rows
  are m).  Per-n score offsets cancel in softmax and are dropped; the v-side
  LN bias folds into the proj bias on the host.
- exp is split across engines: ACT (exact, with per-partition scale),
  DVE (4-op cubic minimax poly on [-0.95, 0.95]), GPSIMD (4-op cubic from a
  DVE-staged prescaled bf16 copy).  Scores live in [-0.71, 0.68].
- Emission order starts ACT exp as early as possible: q -> conv(x2) -> k2 ->
  all nh2=0 scores+exp -> conv(x1)+v1 -> nh2=0 U/pden -> normalize ->
  nh2=1 pipelined loop.
- PSUM budget (8 banks): cv[128,512]x2, scA[128,1024]x2, U0, U1; pden rides
  the cv ring after all phase-1 allocations.
"""

import numpy as np

B, N, C = 2, 4096, 256
H, Dh = 8, 32
M = 1024          # (64/2) * (64/2)
NCH = 1024        # query rows per core
SCALE = Dh ** -0.5
EPS = 1e-5

# minimax cubic fit of exp on [-0.95, 0.95] (rel err <= 4.1e-3 fp32)
C3, C2, C1, C0 = 0.15927659, 0.53526688, 1.00884709, 0.99703789

_prog_cache = {}


def _exp_engine(t):
    # t in [0, 64): tile index in emission order. Returns 'act'|'dve'.
    if t % 6 == 1:
        return "dve"
    return "act"


def _build_program():
    import concourse.bass as bass
    import concourse.bacc as bacc
    import concourse.tile as tile
    from concourse import mybir

    f32 = mybir.dt.float32
    bf16 = mybir.dt.bfloat16
    AF = mybir.ActivationFunctionType
    OP = mybir.AluOpType

    nc = bacc.Bacc()

    x1t = nc.dram_tensor("x1t", [C, N], bf16, kind="ExternalInput")
    x2t = nc.dram_tensor("x2t", [C, N], bf16, kind="ExternalInput")
    xqt = nc.dram_tensor("xqt", [C, NCH], bf16, kind="ExternalInput")
    w2d = nc.dram_tensor("w2", [2, 2, C, C], bf16, kind="ExternalInput")
    wall = nc.dram_tensor("wall", [C, 4 * C], bf16, kind="ExternalInput")
    rowd = nc.dram_tensor("rowd", [2, C], bf16, kind="ExternalInput")   # -kfcol, -vfcol
    cold = nc.dram_tensor("cold", [128, 4], f32, kind="ExternalInput")  # srb(2 oh), pb(2 oh)
    blkd = nc.dram_tensor("blkd", [2, 128, 128], bf16, kind="ExternalInput")
    eyed = nc.dram_tensor("eyed", [8, 8], f32, kind="ExternalInput")
    rstd = nc.dram_tensor("rstd", [2, M], f32, kind="Internal")
    outt = nc.dram_tensor("outt", [C, NCH], f32, kind="ExternalOutput")
    import os
    _dbg = os.environ.get("KDBG", "0") == "1"
    if _dbg:
        dbg_mu = nc.dram_tensor("dbg_mu", [2, M], f32, kind="ExternalOutput")
        dbg_rs = nc.dram_tensor("dbg_rs", [2, M], f32, kind="ExternalOutput")
        dbg_rc = nc.dram_tensor("dbg_rc", [2, 128, 8], f32, kind="ExternalOutput")
        dbg_k2 = nc.dram_tensor("dbg_k2", [128, 128], f32, kind="ExternalOutput")
        dbg_v1 = nc.dram_tensor("dbg_v1", [128, 128], f32, kind="ExternalOutput")
        dbg_q = nc.dram_tensor("dbg_q", [128, 128], f32, kind="ExternalOutput")
        dbg_et = nc.dram_tensor("dbg_et", [128, 1024], f32, kind="ExternalOutput")

    with nc.allow_low_precision(reason="bf16 matmul inputs; fp32 PSUM accumulation"), \
         tile.TileContext(nc) as tc:
      with tc.tile_pool(name="pg", bufs=1) as PG, \
           tc.tile_pool(name="psum", bufs=1, space="PSUM") as PS:
        # ================= consts / weights =================
        wallt = [PG.tile([128, 4 * C], bf16, name=f"wall{ch}",
                         tag=f"wall{ch}") for ch in range(2)]
        nc.gpsimd.dma_start(out=wallt[0], in_=wall[0:128, :])
        nc.sync.dma_start(out=wallt[1], in_=wall[128:256, :])
        wsb = {}
        for wi, nm in enumerate(("q", "k", "v", "p")):
            for ch in range(2):
                wsb[(nm, ch)] = wallt[ch][:, wi * C:(wi + 1) * C]
        kfneg = PG.tile([1, C], bf16, name="kfneg", tag="kfneg")
        nc.gpsimd.dma_start(out=kfneg, in_=rowd[0:1, :])
        vfneg = PG.tile([1, C], bf16, name="vfneg", tag="vfneg")
        nc.gpsimd.dma_start(out=vfneg, in_=rowd[1:2, :])
        cols = PG.tile([128, 4], f32, name="cols", tag="cols")
        nc.gpsimd.dma_start(out=cols, in_=cold[:])
        blk = []
        for grp in range(2):
            t = PG.tile([128, 128], bf16, name=f"blk{grp}", tag=f"blk{grp}")
            nc.gpsimd.dma_start(out=t, in_=blkd[grp])
            blk.append(t)
        w2 = []
        for ch in range(2):
            t = PG.tile([128, 2, 2, C], bf16, name=f"w2{ch}", tag=f"w2{ch}")
            nc.scalar.dma_start(
                out=t,
                in_=w2d[:, :, ch * 128:(ch + 1) * 128, :].rearrange(
                    "kh kw c o -> c kh kw o"))
            w2.append(t)
        xq = []
        for ch in range(2):
            t = PG.tile([128, NCH], bf16, name=f"xq{ch}", tag=f"xq{ch}")
            nc.scalar.dma_start(out=t, in_=xqt[ch * 128:(ch + 1) * 128, :])
            xq.append(t)
        # x loads split in N-halves so conv can start on the first half;
        # x2 (k2 path) first, spread over queues.
        xT = {}
        for inp, dram in ((1, x2t), (0, x1t)):
            for ch in range(2):
                t = PG.tile([128, N], bf16, name=f"x{inp}{ch}", tag=f"x{inp}{ch}")
                xT[(inp, ch)] = t
        for half in range(2):
            hs = slice(half * 2048, (half + 1) * 2048)
            nc.gpsimd.dma_start(out=xT[(1, 0)][:, hs], in_=x2t[0:128, hs])
            nc.sync.dma_start(out=xT[(1, 1)][:, hs], in_=x2t[128:256, hs])
        for half in range(2):
            hs = slice(half * 2048, (half + 1) * 2048)
            nc.gpsimd.dma_start(out=xT[(0, 0)][:, hs], in_=x1t[0:128, hs])
            nc.sync.dma_start(out=xT[(0, 1)][:, hs], in_=x1t[128:256, hs])
        # small consts after the bulk loads (gpsimd queue is in order)

        ones1 = PG.tile([1, 128], bf16, name="ones1", tag="ones1")
        nc.vector.memset(ones1, 1.0)
        selc = PG.tile([128, 1], bf16, name="selc", tag="selc")
        nc.vector.memset(selc, 1.0)
        sel8 = PG.tile([128, 2, 8], bf16, name="sel8", tag="sel8")
        nc.vector.memset(sel8, 0.0)
        for g in range(2):
            nc.vector.memset(sel8[:, g, g:g + 1], 1.0)
        eye8 = PG.tile([8, 8], f32, name="eye8", tag="eye8")
        nc.gpsimd.dma_start(out=eye8, in_=eyed[:])
        epsT = PG.tile([1, 1], f32, name="epsT", tag="epsT")
        nc.vector.memset(epsT, EPS)

        # ================= SBUF data tiles =================
        qT = [PG.tile([128, NCH], bf16, name=f"qT{oh}", tag=f"qT{oh}")
              for oh in range(2)]
        xr = {(inp, oh): PG.tile([128, M], bf16, name=f"xr{inp}{oh}",
                                 tag=f"xr{inp}{oh}")
              for inp in range(2) for oh in range(2)}
        # sq tiles shared between the two inputs (x2 stats finish before
        # conv(x1) writes them again; WAR handled by tile deps)
        sq = {}
        for inp in (1, 0):   # allocation order must match usage order
            for oh in range(2):
                sq[(inp, oh)] = PG.tile([128, M], bf16, name=f"sqt{oh}",
                                        tag=f"sqt{oh}", bufs=1)
        k2x = [PG.tile([128, M], bf16, name=f"k2x{oh}", tag=f"k2x{oh}")
               for oh in range(2)]
        v1 = [PG.tile([128, C], bf16, name=f"v1_{ms}", tag=f"v1_{ms}")
              for ms in range(8)]
        murow = [PG.tile([1, M], bf16, name=f"mu{inp}", tag=f"mu{inp}")
                 for inp in range(2)]
        varrow = [PG.tile([1, M], f32, name=f"va{inp}", tag=f"va{inp}")
                  for inp in range(2)]
        rcol = [PG.tile([128, 8], f32, name=f"rcol{inp}", tag=f"rcol{inp}")
                for inp in range(2)]

        def cvt():
            return PS.tile([128, 512], f32, name="cv", tag="cv", bufs=2)

        # ================= q (scA ring; interleaved with conv below) ====
        def q_tile(oh, nh):
            ps = PS.tile([128, 512], f32, name="qp", tag="scA", bufs=2)
            for ch in range(2):
                nc.tensor.matmul(
                    ps, wsb[("q", ch)][:, oh * 128:(oh + 1) * 128],
                    xq[ch][:, nh * 512:(nh + 1) * 512],
                    start=(ch == 0), stop=(ch == 1))
            nc.scalar.copy(out=qT[oh][:, nh * 512:(nh + 1) * 512], in_=ps)

        # ================= conv + stats + proj-k/v helpers ============
        def conv_tile(inp, mh, oh):
            # conv for m-columns [mh*512, (mh+1)*512) of one oh chunk
            if True:
                ps = cvt()
                k = 0
                for ch in range(2):
                    xv = xT[(inp, ch)].rearrange(
                        "p (i ki j kj) -> p ki kj i j", ki=2, kj=2, j=32)
                    for kh in range(2):
                        for kw in range(2):
                            nc.tensor.matmul(
                                ps,
                                w2[ch][:, kh, kw, oh * 128:(oh + 1) * 128],
                                xv[:, kh, kw, mh * 16:(mh + 1) * 16, :],
                                start=(k == 0), stop=(k == 7))
                            k += 1
                sl = slice(mh * 512, (mh + 1) * 512)
                # copy + per-partition conv bias (srb col 0/1)
                nc.vector.tensor_scalar(
                    out=xr[(inp, oh)][:, sl], in0=ps,
                    scalar1=cols[:, oh:oh + 1], scalar2=None, op0=OP.add)
                nc.gpsimd.tensor_mul(sq[(inp, oh)][:, sl],
                                     xr[(inp, oh)][:, sl],
                                     xr[(inp, oh)][:, sl])

        def conv_half(inp, mh):
            for oh in range(2):
                conv_tile(inp, mh, oh)

        def stats_half(inp, mh):
            sl = slice(mh * 512, (mh + 1) * 512)
            pmu = PS.tile([1, 512], f32, name="pmu", tag="scA", bufs=2)
            psq = PS.tile([1, 512], f32, name="psq", tag="scA", bufs=2)
            for k, oh in enumerate(range(2)):
                nc.tensor.matmul(pmu, selc, xr[(inp, oh)][:, sl],
                                 start=(k == 0), stop=(k == 1))
                nc.tensor.matmul(psq, selc, sq[(inp, oh)][:, sl],
                                 start=(k == 0), stop=(k == 1))
            nc.scalar.mul(out=murow[inp][:, sl], in_=pmu, mul=1.0 / C)
            mu2 = PG.tile([1, 512], f32, name="mu2", tag="mu2", bufs=2)
            nc.vector.tensor_mul(mu2, murow[inp][:, sl], murow[inp][:, sl])
            nc.vector.scalar_tensor_tensor(
                out=varrow[inp][:, sl], in0=psq, scalar=1.0 / C, in1=mu2,
                op0=OP.mult, op1=OP.subtract)

        def rcol_make(inp, mh):
            # var row half [1, 512] -> [128, 4] columns (DRAM bounce), then
            # rsqrt(var + eps) on DVE: bit-trick seed + 2 Newton steps.
            sl = slice(mh * 512, (mh + 1) * 512)
            cs = slice(mh * 4, (mh + 1) * 4)
            nc.gpsimd.dma_start(out=rstd[inp:inp + 1, sl],
                                in_=varrow[inp][:, sl])
            vc = PG.tile([128, 4], f32, name="vc", tag="vc", bufs=2)
            nc.gpsimd.dma_start(
                out=vc,
                in_=rstd[inp:inp + 1, sl].rearrange("o (j p) -> (o p) j",
                                                    p=128))
            nc.vector.tensor_scalar(out=vc, in0=vc, scalar1=float(EPS),
                                    scalar2=None, op0=OP.add)
            i32 = mybir.dt.int32
            sh = PG.tile([128, 4], i32, name="sh", tag="sh", bufs=2)
            nc.vector.tensor_scalar(out=sh, in0=vc.bitcast(i32), scalar1=1,
                                    scalar2=None, op0=OP.arith_shift_right)
            y0 = PG.tile([128, 4], i32, name="y0", tag="y0", bufs=2)
            nc.vector.tensor_scalar(out=y0, in0=sh, scalar1=-1,
                                    scalar2=0x5F3759DF,
                                    op0=OP.mult, op1=OP.add)
            y = y0.bitcast(f32)
            for it in range(2):
                c = PG.tile([128, 4], f32, name="nc1", tag="nc1", bufs=2)
                nc.vector.tensor_mul(c, y, y)
                nc.vector.tensor_mul(c, c, vc)
                nc.vector.tensor_scalar(out=c, in0=c, scalar1=-0.5,
                                        scalar2=1.5, op0=OP.mult, op1=OP.add)
                dst = rcol[inp][:, cs] if it == 1 else y
                nc.vector.tensor_mul(dst, y, c)

        def k2x_half(mh):
            sl = slice(mh * 512, (mh + 1) * 512)
            for oh in range(2):
                ps = cvt()
                for ch in range(2):
                    nc.tensor.matmul(
                        ps, wsb[("k", ch)][:, oh * 128:(oh + 1) * 128],
                        xr[(1, ch)][:, sl], start=(ch == 0), stop=False)
                # rank-1: += (-kfcol) x mu2  (mean subtraction folded in)
                nc.tensor.matmul(
                    ps, kfneg[:, oh * 128:(oh + 1) * 128],
                    murow[1][:, sl], start=False, stop=True)
                nc.vector.tensor_copy(out=k2x[oh][:, sl], in_=ps)

        def v1_chunk(ms):
            msl = slice(ms * 128, (ms + 1) * 128)
            ps = PS.tile([128, C], f32, name="vp", tag="cv", bufs=2)
            for ch in range(2):
                nc.tensor.matmul(ps, xr[(0, ch)][:, msl], wsb[("v", ch)],
                                 start=(ch == 0), stop=False)
            # rank-1: += mu1 x (-vfcol)
            nc.tensor.matmul(ps, murow[0][:, msl], vfneg,
                             start=False, stop=True)
            # copy with per-partition rstd scale folded in
            nc.vector.tensor_scalar(
                out=v1[ms], in0=ps, scalar1=rcol[0][:, ms:ms + 1],
                scalar2=None, op0=OP.mult)

        # ================= phase 2 machinery =================
        ET = tc.alloc_tile_pool(name="et", bufs=(28 if _dbg else 34))
        XG = tc.alloc_tile_pool(name="xg", bufs=3)
        tile_ctr = [0]

        def scores_exp(nh2, ms, grp):
            """scores for 2 pr-tiles (4 heads) + exp on assigned engine.
            Returns the two et tiles."""
            nsl = slice(nh2 * 512, (nh2 + 1) * 512)
            ets = []
            for pr in range(2):
                t = tile_ctr[0]
                tile_ctr[0] += 1
                eng = _exp_engine(t)
                scps = PS.tile([128, 1024], f32, name="scps", tag="scA",
                               bufs=2)
                for i in range(2):
                    h = grp * 4 + pr * 2 + i
                    hb = 32 * (h % 4)
                    nc.tensor.matmul(
                        scps[:, i * 512:(i + 1) * 512],
                        k2x[h // 4][hb:hb + 32, ms * 128:(ms + 1) * 128],
                        qT[h // 4][hb:hb + 32, nsl],
                        start=True, stop=True,
                        tile_position=(hb, 0))
                et = ET.tile([128, 1024], bf16, name="et", tag="et")
                rsc = rcol[1][:, ms:ms + 1]
                if eng == "act":
                    nc.scalar.activation(out=et, in_=scps, func=AF.Exp,
                                         scale=rsc)
                else:  # dve cubic
                    t = XG.tile([128, 1024], bf16, name="t", tag="t", bufs=1)
                    nc.vector.tensor_scalar(
                        out=t, in0=scps, scalar1=rsc, scalar2=None,
                        op0=OP.mult)
                    u = XG.tile([128, 1024], bf16, name="u", tag="u", bufs=1)
                    nc.vector.tensor_mul(u, t, t)
                    v = XG.tile([128, 1024], bf16, name="v", tag="v", bufs=1)
                    nc.vector.tensor_scalar(
                        out=v, in0=t, scalar1=float(C3), scalar2=float(C2),
                        op0=OP.mult, op1=OP.add)
                    p = XG.tile([128, 1024], bf16, name="p", tag="p", bufs=1)
                    nc.vector.tensor_mul(p, u, v)
                    w = XG.tile([128, 1024], bf16, name="w", tag="w", bufs=1)
                    nc.vector.tensor_scalar(
                        out=w, in0=t, scalar1=float(C1), scalar2=float(C0),
                        op0=OP.mult, op1=OP.add)
                    nc.vector.tensor_add(et, w, p)
                ets.append(et)
            return ets

        U = {}
        pden = {}

        def upden(nh2, ms, grp, ets):
            if ms == 0 and grp == 0:
                U[(nh2, 0)] = PS.tile([128, 512], f32, name="U0", tag="U0")
                U[(nh2, 1)] = PS.tile([128, 512], f32, name="U1", tag="U1")
                pden[nh2] = PS.tile([128, 512], f32, name="pden", tag="cv",
                                    bufs=2)
            for pr in range(2):
                for i in range(2):
                    h = grp * 4 + pr * 2 + i
                    h4 = pr * 2 + i
                    esl = ets[pr][:, i * 512:(i + 1) * 512]
                    nc.tensor.matmul(
                        U[(nh2, grp)][32 * h4:32 * h4 + 32, :],
                        v1[ms][:, 32 * h:32 * h + 32], esl,
                        start=(ms == 0), stop=(ms == 7),
                        tile_position=(0, 32 * h4),
                        skip_group_check=True)
            for pr in range(2):
                for i in range(2):
                    h = grp * 4 + pr * 2 + i
                    g = h % 4
                    esl = ets[pr][:, i * 512:(i + 1) * 512]
                    nc.tensor.matmul(
                        pden[nh2][32 * g:32 * g + 8, :],
                        sel8[:, h // 4, :], esl,
                        start=(ms == 0 and grp == 0),
                        stop=(ms == 7 and grp == 1),
                        tile_position=(0, 32 * g),
                        skip_group_check=True)

        def normalize_proj(nh2):
            nsl = slice(nh2 * 512, (nh2 + 1) * 512)
            pdenS = PG.tile([128, 512], bf16, name="pdenS", tag="pdenS",
                            bufs=2)
            nc.vector.tensor_copy(out=pdenS, in_=pden[nh2])
            rps = PS.tile([128, 1024], f32, name="rps", tag="scA", bufs=2)
            for grp in range(2):
                nc.tensor.matmul(rps[:, grp * 512:(grp + 1) * 512],
                                 blk[grp], pdenS, start=True, stop=True)
            recf = PG.tile([128, 1024], f32, name="recf", tag="recf",
                           bufs=2)
            nc.vector.reciprocal_approx_fast(out=recf, in_=rps)
            oT = []
            for grp in range(2):
                ot = PG.tile([128, 512], bf16, name="ot", tag=f"ot{grp}",
                             bufs=2)
                nc.vector.tensor_mul(ot, U[(nh2, grp)],
                                     recf[:, grp * 512:(grp + 1) * 512])
                oT.append(ot)
            for oh in range(2):
                ps = cvt()
                for ch in range(2):
                    nc.tensor.matmul(
                        ps, wsb[("p", ch)][:, oh * 128:(oh + 1) * 128],
                        oT[ch], start=(ch == 0), stop=(ch == 1))
                y = PG.tile([128, 512], f32, name="y", tag="y", bufs=2)
                # copy + per-partition proj bias (pb col 2/3)
                nc.vector.tensor_scalar(
                    out=y, in0=ps, scalar1=cols[:, 2 + oh:3 + oh],
                    scalar2=None, op0=OP.add)
                nc.gpsimd.dma_start(out=outt[oh * 128:(oh + 1) * 128, nsl],
                                    in_=y)

        # ================= emission =================
        # phase 1a: q and conv(x2) interleaved (independent PSUM rings),
        # then per-half stats -> rcol -> k2x so scores can start after the
        # first half.
        q_tile(0, 0)
        conv_tile(1, 0, 0)
        q_tile(1, 0)
        conv_tile(1, 0, 1)
        stats_half(1, 0)
        rcol_make(1, 0)
        k2x_half(0)
        q_tile(0, 1)
        conv_tile(1, 1, 0)
        q_tile(1, 1)
        conv_tile(1, 1, 1)
        stats_half(1, 1)
        rcol_make(1, 1)
        k2x_half(1)

        # phase 2a: nh2=0 scores + exp, with phase 1b (conv(x1)+v1) steps
        # interleaved so the PE stays dense while ACT chews on exp.
        p1b_steps = (
            [lambda: conv_tile(0, 0, 0), lambda: conv_tile(0, 0, 1),
             lambda: stats_half(0, 0), lambda: rcol_make(0, 0),
             lambda: conv_tile(0, 1, 0), lambda: conv_tile(0, 1, 1),
             lambda: stats_half(0, 1), lambda: rcol_make(0, 1)] +
            [lambda ms=ms: v1_chunk(ms) for ms in range(8)]
        )
        ets0 = {}
        step = 0
        for ms in range(8):
            for grp in range(2):
                ets0[(ms, grp)] = scores_exp(0, ms, grp)
                p1b_steps[step]()
                step += 1

        if _dbg:
            for inp in range(2):
                mcf = PG.tile([1, M], f32, name="mcf", tag="mcf", bufs=1)
                nc.vector.tensor_copy(out=mcf, in_=murow[inp])
                nc.gpsimd.dma_start(out=dbg_mu[inp:inp+1, :], in_=mcf)
                nc.gpsimd.dma_start(out=dbg_rs[inp:inp+1, :], in_=rsrow[inp])
                nc.gpsimd.dma_start(out=dbg_rc[inp], in_=rcol[inp])
            smp = PG.tile([128, 128], f32, name="smp", tag="smp", bufs=2)
            for nmd, srcd in (("dbg_k2", k2x[0]), ("dbg_v1", v1[0]),
                              ("dbg_q", qT[0])):
                s = PG.tile([128, 128], f32, name="s_" + nmd, tag="smp",
                            bufs=2)
                nc.vector.tensor_copy(out=s, in_=srcd[:, 0:128])
                d = {"dbg_k2": dbg_k2, "dbg_v1": dbg_v1, "dbg_q": dbg_q}[nmd]
                nc.gpsimd.dma_start(out=d[:, 0:128], in_=s)
            etf = PG.tile([128, 1024], f32, name="etf", tag="etf")
            nc.vector.tensor_copy(out=etf, in_=ets0[(0, 0)][0])
            nc.gpsimd.dma_start(out=dbg_et[:], in_=etf)

        # phase 2b: interleave nh2=1 scores+exp with nh2=0 U/pden so the
        # ACT engine never starves while the PE drains the stored et tiles
        ets1 = {}
        for ms in range(8):
            for grp in range(2):
                ets1[(ms, grp)] = scores_exp(1, ms, grp)
                upden(0, ms, grp, ets0[(ms, grp)])
        normalize_proj(0)

        # phase 2c: nh2=1 U/pden
        for ms in range(8):
            for grp in range(2):
                upden(1, ms, grp, ets1[(ms, grp)])
        normalize_proj(1)

        XG.release()
        ET.release()
    nc.finalize()
    return nc


def _get_program():
    if "nc" not in _prog_cache:
        _prog_cache["nc"] = _build_program()
    return _prog_cache["nc"]


def kernel(x1, x2, q_w, kv_w, sr_w, sr_b, ln_g, ln_b, proj_w, proj_b,
           H1=64, W1=64, H2=64, W2=64, **_):
    from concourse.bass_utils import run_bass_kernel_spmd

    f = np.float32
    x1 = np.asarray(x1, f)
    x2 = np.asarray(x2, f)
    q_w = np.asarray(q_w, f)
    kv_w = np.asarray(kv_w, f)
    sr_w = np.asarray(sr_w, f)
    sr_b = np.asarray(sr_b, f)
    ln_g = np.asarray(ln_g, f)
    ln_b = np.asarray(ln_b, f)
    proj_w = np.asarray(proj_w, f)
    proj_b = np.asarray(proj_b, f)

    import ml_dtypes
    bf = ml_dtypes.bfloat16

    qwT = np.ascontiguousarray(q_w.T * SCALE)
    kwTf = np.ascontiguousarray(ln_g[:, None] * kv_w[:C].T)   # [cin, out]
    vwTf = np.ascontiguousarray(ln_g[:, None] * kv_w[C:].T)
    kfcol_neg = -kwTf.sum(axis=0)    # [C]
    vfcol_neg = -vwTf.sum(axis=0)
    bvec_k = kv_w[:C] @ ln_b         # dropped: constant along m, cancels
    bvec_v = kv_w[C:] @ ln_b
    pbias = proj_b + proj_w @ bvec_v
    pwT = np.ascontiguousarray(proj_w.T)
    w2 = np.ascontiguousarray(sr_w.transpose(2, 3, 1, 0))
    rowd = np.stack([kfcol_neg, vfcol_neg], axis=0)           # [2, C]
    cold = np.stack([sr_b[:128], sr_b[128:],
                     pbias[:128], pbias[128:]], axis=1)       # [128, 4]
    blkd = np.zeros((2, 128, 128), bf)
    for grp in range(2):
        for i in range(128):
            h = grp * 4 + i // 32
            src_row = 32 * (h % 4) + h // 4
            blkd[grp, src_row, i] = 1.0

    x1T = [np.ascontiguousarray(x1[b].T).astype(bf) for b in range(B)]
    x2T = [np.ascontiguousarray(x2[b].T).astype(bf) for b in range(B)]

    in_maps = []
    for core in range(8):
        b, chk = divmod(core, 4)
        in_maps.append({
            "x1t": x1T[b], "x2t": x2T[b],
            "xqt": np.ascontiguousarray(x1T[b][:, chk * NCH:(chk + 1) * NCH]),
            "w2": w2.astype(bf),
            "wall": np.ascontiguousarray(
                np.concatenate([qwT, kwTf, vwTf, pwT], axis=1)).astype(bf),
            "rowd": rowd.astype(bf), "cold": cold.astype(np.float32),
            "blkd": blkd, "eyed": np.eye(8, dtype=np.float32),
        })

    nc = _get_program()
    res = run_bass_kernel_spmd(nc, in_maps, core_ids=list(range(8)))
    _prog_cache["last_result"] = res
    out = np.empty((B, N, C), f)
    for core in range(8):
        b, chk = divmod(core, 4)
        out[b, chk * NCH:(chk + 1) * NCH, :] = res.results[core]["outt"].T
    return out
